# revision 1
# baseline (speedup 1.0000x reference)
"""Trainium2 Bass kernel for nn_BasicBlock_1w8a_q (quantized ResNet BasicBlock,
1-bit weights / 8-bit activations).

Strategy:
 - Pure data parallel over 8 NeuronCores: batch 32 -> 4 images per core.
 - Layout: channels C=128 on SBUF partitions, spatial on the free dim.
 - Each 3x3 conv with sign(+-1) weights = 9 shifted matmuls accumulating in
   PSUM (lhsT = [C_in, C_out] weight slice, rhs = shifted padded input view).
 - /4 is folded into the weights (+-0.25, exact in fp16; power-of-2 scaling
   commutes with IEEE rounding so psum == conv/4 bit-for-bit).
 - conv1 runs in 2 fp16 passes (x = hi + lo split, ~f32-exact);
   conv2 runs in 1 fp16 pass (x1 is integers in [-7,7]: exact).
 - Rounds use the DVE f32->int32 cast (exact round-half-to-even, verified on
   HW, == jnp.round) or the +1.5*2^23 magic-constant trick (fp32 add rounds
   to integer by RNE), which lets the ScalarE (ACT) do rounds too.
 - BN folding / per-channel constants are computed on host mirroring the
   reference's f32 op order; data-dependent fused scales are grid-verified
   on host against the reference mapping before use.
"""

import os

import numpy as np

import concourse.bass as bass
import concourse.bacc as bacc
import concourse.tile as tile
import concourse.mybir as mybir
from concourse.bass_utils import run_bass_kernel_spmd
from concourse.mybir import AluOpType as Op

F32 = mybir.dt.float32
F16 = mybir.dt.float16
I32 = mybir.dt.int32
I8 = mybir.dt.int8
F8 = mybir.dt.float8e4
WP8 = 64                    # fp8 x1 row pitch (pair stride must be %16==0)
IDENT = mybir.ActivationFunctionType.Identity

B, C, H, W = 32, 128, 56, 56
NCORES = 8
BS = B // NCORES            # images per core
HP, WP = H + 2, W + 2       # padded spatial
HB = 8                      # output rows per psum chunk
NCH = H // HB               # chunks per image (7)
CHUNK = HB * W              # 448 columns per psum chunk
BANK = 512                  # fp32 slots per PSUM bank
GROUPS = [(0, 4), (4, 3)]   # (first chunk, n chunks) per psum group
SHIFTS = [(ky, kx) for ky in range(3) for kx in range(3)]
MAGIC = float(np.float32(12582912.0))   # 1.5 * 2^23, even integer

f32 = np.float32


# ---------------------------------------------------------------------------
# Host-side prep: mirrors the reference's f32 op order exactly.
# ---------------------------------------------------------------------------

def _qfn(x, prec):
    n = f32(2.0 ** prec - 1.0)
    q = (np.round(x * n) / n).astype(f32)
    return (x + (q - x)).astype(f32)


def _my_quantize(x, prec):
    T = np.clip(np.max(np.abs(x)), f32(1e-10), f32(255.0)).astype(f32)
    return (_qfn((np.clip(x, -T, T) / T).astype(f32), prec) * T).astype(f32)


def _bn_consts(gamma, beta, mean, var):
    gamma, beta, mean, var = (a.astype(f32) for a in (gamma, beta, mean, var))
    std = np.sqrt(var + f32(1e-5)).astype(f32)
    w = (gamma / std).astype(f32)
    bq = (beta - w * mean).astype(f32)
    T_w = np.max(np.abs(w)).astype(f32)
    bw = (_qfn((np.clip(w, -T_w, T_w) / T_w).astype(f32), 3) * f32(7.0)).astype(f32)
    qb = _my_quantize(bq, 14)
    t = (qb * f32(7.0)).astype(f32)
    t = (t * f32(1023.0)).astype(f32)
    t = (t / f32(4032.0)).astype(f32)
    t = (t * f32(7.0)).astype(f32)
    t = (t / T_w).astype(f32)
    bb = np.round(t).astype(f32)
    return bw, bb, T_w


def _sc_th(T_w):
    a = (f32(1023.0) / f32(4032.0)).astype(f32)
    a = (a * f32(7.0)).astype(f32)
    sc = np.round((a / T_w).astype(f32)).astype(f32)
    b2 = (f32(7.0) * f32(1023.0)).astype(f32)
    b2 = (b2 / f32(4032.0)).astype(f32)
    b2 = (b2 * f32(7.0)).astype(f32)
    Th = np.round((b2 / T_w).astype(f32)).astype(f32)
    return sc, Th


def _ref_final_vec(k, Th):
    # reference: round(clip(k,-Th,Th)/Th*7.0) elementwise in f32
    kk = np.clip(k.astype(f32), -Th, Th).astype(f32)
    return np.round(((kk / Th).astype(f32) * f32(7.0)).astype(f32))


def _scale_cands(Th):
    base = f32(f32(7.0) / f32(Th))
    out = [base]
    up, dn = base, base
    for _ in range(8):
        up = np.nextafter(up, f32(np.inf), dtype=f32)
        dn = np.nextafter(dn, f32(-np.inf), dtype=f32)
        out += [up, dn]
    return out


def _pick_scale(Th):
    """s (f32) with clip(RNE(k*s),-7,7) == round(clip(k,-Th,Th)/Th*7) for all
    integer k (device RNE == np.round, verified on HW)."""
    kk = np.arange(-3000, 3001, dtype=f32)
    want = _ref_final_vec(kk, f32(Th))
    for s in _scale_cands(Th):
        got = np.clip(np.round((kk * s).astype(f32)), -7.0, 7.0)
        if np.array_equal(got, want):
            return f32(s)
    raise AssertionError(f"no matching scale for Th={Th}")


def _pick_fused_stage2(bw2, bb2, sc2, Th2):
    """Stage-2 fusion: u2 = RNE(t2*(bw2*s) + (x1*(sc2*s) + bb2*s)) must equal
    ref round(clip(v2)/Th2*7) (then clip +-7) for v2 = t2*bw2 + x1*sc2 + bb2.
    Returns (B2s, rscale, rbias, s) all f32, host-verified over a full grid
    with a tie-margin so ACT fma-vs-two-round ambiguity cannot flip a round.
    """
    t2g = np.arange(-640, 641, dtype=f32)[None, :, None]       # [1,T,1]
    x1g = np.arange(-7, 8, dtype=f32)[None, None, :]           # [1,1,15]
    bwc = bw2.astype(f32)[:, None, None]                       # [C,1,1]
    bbc = bb2.astype(f32)[:, None, None]
    v2 = (t2g * bwc + x1g * f32(sc2) + bbc).astype(f32)        # exact ints
    want = np.clip(_ref_final_vec(v2, f32(Th2)), -7.0, 7.0)
    base = f32(f32(7.0) / f32(Th2))
    for j in range(0, 60):
        s = f32(base * f32(1.0 + j * 2.0 ** -19))
        B2s = (bw2 * s).astype(f32)
        rscale = f32(f32(sc2) * s)
        rbias = (bb2 * s).astype(f32)
        # device sim (two-round form)
        r2s = ((x1g * rscale).astype(f32) + rbias[:, None, None]).astype(f32)
        dev = ((t2g * B2s[:, None, None]).astype(f32) + r2s).astype(f32)
        got = np.clip(np.round(dev), -7.0, 7.0)
        if not np.array_equal(got, want):
            continue
        # tie-margin: exact value far enough from half-integers (so device
        # fma-vs-two-round differences, bounded ~6e-6 abs in-range, cannot
        # flip a round) unless the result saturates either way
        z = (t2g.astype(np.float64) * B2s.astype(np.float64)[:, None, None]
             + x1g.astype(np.float64) * float(rscale)
             + rbias.astype(np.float64)[:, None, None])
        dist = np.abs(z - (np.floor(z) + 0.5))
        safe = (dist > 3e-5) | (np.abs(z) > 7.6)
        if bool(np.all(safe)):
            return B2s, rscale, rbias, f32(s)
    raise AssertionError(f"no verified fused scale for Th2={Th2}")


def _host_prep(x, w1, w2, g1, b1, m1, v1, g2, b2, m2, v2):
    w1 = w1.astype(f32)
    w2 = w2.astype(f32)
    sw1 = np.abs(w1).mean(axis=(1, 2, 3), dtype=np.float32).astype(f32)
    sw2 = np.abs(w2).mean(axis=(1, 2, 3), dtype=np.float32).astype(f32)
    bw1, bb1, Tw1 = _bn_consts(g1, b1, m1, v1)
    bw2, bb2, Tw2 = _bn_consts(g2, b2, m2, v2)
    sc1, Th1 = _sc_th(Tw1)
    sc2, Th2 = _sc_th(Tw2)
    s1 = _pick_scale(Th1)
    B2s, rscale, rbias, _s2 = _pick_fused_stage2(bw2, bb2, sc2, Th2)

    def wtiles(w):
        sg = (np.sign(w) * 0.25).astype(np.float16)  # [O, I, 3, 3]
        t = np.empty((C, 9, C), np.float16)          # [ci, s, co]
        for s, (ky, kx) in enumerate(SHIFTS):
            t[:, s, :] = sg[:, :, ky, kx].T
        return t

    def wtiles8(w, scale=0.25):
        np8 = mybir.dt.np(F8)
        sg = (np.sign(w) * scale).astype(np.float32)  # [O, I, 3, 3]
        d = np.empty((C, 3, 2, C), np.float32)       # [ci, kx, ky(0,1), co]
        r = np.empty((C, 3, C), np.float32)          # [ci, kx, co] (ky=2)
        for kx in range(3):
            d[:, kx, 0, :] = sg[:, :, 0, kx].T
            d[:, kx, 1, :] = sg[:, :, 1, kx].T
            r[:, kx, :] = sg[:, :, 2, kx].T
        return d.astype(np8), r.astype(np8)

    cv = np.zeros((C, 12), f32)
    cv[:, 0] = sw1                       # A1
    cv[:, 1] = bw1                       # B1
    cv[:, 2] = bb1 + f32(MAGIC)          # bb1 + C (exact: bb1 int, C int)
    cv[:, 3] = sc1                       # sc1 (broadcast)
    cv[:, 4] = s1                        # s1 (broadcast)
    cv[:, 5] = sw2                       # A2
    cv[:, 6] = B2s                       # bw2 * s2
    cv[:, 7] = rscale                    # sc2 * s2 (broadcast)
    cv[:, 8] = rbias                     # bb2 * s2
    cv[:, 9] = f32(MAGIC)                # +C for ACT rounding
    cv[:, 10] = f32(-MAGIC)              # -C
    w1d, w1r = wtiles8(w1, scale=2.0 ** -8)
    w2d, w2r = wtiles8(w2)
    return wtiles(w1), w1d, w1r, w2d, w2r, cv


# ---------------------------------------------------------------------------
# Device program
# ---------------------------------------------------------------------------

_prog_cache = {}


def _build_program():
    passes = int(os.environ.get("CONV1_PASSES", "2"))
    lo_mode = os.environ.get("CONV1_LO", "fp16")
    key = ("nc", passes, lo_mode)
    if key in _prog_cache:
        return _prog_cache[key]
    nc = bacc.Bacc("TRN2", target_bir_lowering=False, debug=False,
                   num_devices=NCORES)
    d_x = nc.dram_tensor("xt", [C, BS, H, W], F32, kind="ExternalInput").ap()
    d_w1 = nc.dram_tensor("w1s", [C, 9, C], F16, kind="ExternalInput").ap()
    d_w1d = nc.dram_tensor("w1d", [C, 3, 2, C], F8, kind="ExternalInput").ap()
    d_w1r = nc.dram_tensor("w1r", [C, 3, C], F8, kind="ExternalInput").ap()
    d_w2d = nc.dram_tensor("w2d", [C, 3, 2, C], F8, kind="ExternalInput").ap()
    d_w2r = nc.dram_tensor("w2r", [C, 3, C], F8, kind="ExternalInput").ap()
    d_cv = nc.dram_tensor("cv", [C, 12], F32, kind="ExternalInput").ap()
    d_o = nc.dram_tensor("ot", [C, BS, H, W], I8, kind="ExternalOutput").ap()

    with tile.TileContext(nc) as tc:
        with tc.tile_pool(name="const", bufs=1) as const, \
             tc.tile_pool(name="pads", bufs=1) as pads, \
             tc.tile_pool(name="xin", bufs=2) as xin, \
             tc.tile_pool(name="tmp", bufs=1) as tmp, \
             tc.tile_pool(name="outp", bufs=1) as outp, \
             tc.tile_pool(name="psum", bufs=2, space="PSUM") as psum:

            cv = const.tile([C, 12], F32)
            w1 = const.tile([C, 9, C], F16)
            w1d = const.tile([C, 3, 2, C], F8)
            w1r = const.tile([C, 3, C], F8)
            w2d = const.tile([C, 3, 2, C], F8)
            w2r = const.tile([C, 3, C], F8)

            A1, B1, BB1C, SC1, S1 = (cv[:, i:i + 1] for i in range(5))
            A2, B2S, RSC, RBI = (cv[:, i:i + 1] for i in range(5, 9))
            MAGP, MAGN = cv[:, 9:10], cv[:, 10:11]

            two_pass1 = (passes == 2)
            lo8 = (lo_mode == "fp8")
            xh = pads.tile([C, BS, HP, WP], F16)
            xl8 = None
            xl16 = None
            if two_pass1 and lo8:
                xl8 = pads.tile([C, BS, HP, WP8], F8)
            if two_pass1 and not lo8:
                xl16 = pads.tile([C, BS, HP, WP], F16)
            x1p = pads.tile([C, BS, HP, WP8], F8)
            for b in range(BS):
                nc.gpsimd.memset(xh[:, b, 0, :], 0.0)
                nc.gpsimd.memset(xh[:, b, HP - 1, :], 0.0)
                nc.gpsimd.memset(xh[:, b, 1:HP - 1, 0], 0.0)
                nc.gpsimd.memset(xh[:, b, 1:HP - 1, WP - 1], 0.0)
            if xl16 is not None:
                for b in range(BS):
                    nc.gpsimd.memset(xl16[:, b, 0, :], 0.0)
                    nc.gpsimd.memset(xl16[:, b, HP - 1, :], 0.0)
                    nc.gpsimd.memset(xl16[:, b, 1:HP - 1, 0], 0.0)
                    nc.gpsimd.memset(xl16[:, b, 1:HP - 1, WP - 1], 0.0)
            for buf in ((xl8, x1p) if (two_pass1 and lo8) else (x1p,)):
                for b in range(BS):
                    nc.gpsimd.memset(buf[:, b, 0, :], 0.0)
                    nc.gpsimd.memset(buf[:, b, HP - 1, :], 0.0)
                    nc.gpsimd.memset(buf[:, b, 1:HP - 1, 0], 0.0)
                    nc.gpsimd.memset(buf[:, b, 1:HP - 1, WP - 1:], 0.0)

            out_sb = outp.tile([C, BS, H, W], I8)

            # ---- load + hi/lo split (two row-pieces per image, so the
            # first matmuls start as soon as piece 1 of image 0 lands) ----
            PIECES = ((0, 10), (10, 34), (34, H))
            # cv + w1 first: the very first matmul needs w1 loaded;
            # the fp8 weight tiles are only needed slightly later
            nc.sync.dma_start(out=cv, in_=d_cv)
            nc.sync.dma_start(out=w1, in_=d_w1)
            xfs = []
            for b in range(BS):
                xf = xin.tile([C, H, W], F32, tag="xf")
                for r0, r1 in PIECES:
                    nc.sync.dma_start(out=xf[:, r0:r1, :],
                                      in_=d_x[:, b, r0:r1, :])
                if b == 0:
                    nc.sync.dma_start(out=w1d, in_=d_w1d)
                    nc.sync.dma_start(out=w1r, in_=d_w1r)
                    nc.sync.dma_start(out=w2d, in_=d_w2d)
                    nc.sync.dma_start(out=w2r, in_=d_w2r)
                xhs = None
                if two_pass1 and lo8:
                    xhs = xin.tile([C, H, W], F16, tag="xhs")
                for r0, r1 in PIECES:
                    # hi on ACT (any nearest rounding: lo compensates exactly)
                    nc.scalar.activation(
                        out=xh[:, b, 1 + r0:1 + r1, 1:1 + W],
                        in_=xf[:, r0:r1, :], func=IDENT)
                    if two_pass1 and not lo8:
                        nc.vector.scalar_tensor_tensor(
                            out=xl16[:, b, 1 + r0:1 + r1, 1:1 + W],
                            in0=xf[:, r0:r1, :], scalar=1.0,
                            in1=xh[:, b, 1 + r0:1 + r1, 1:1 + W],
                            op0=Op.mult, op1=Op.subtract)
                    if two_pass1 and lo8:
                        # xhs = fp16(x)*64 exactly (pow2 commutes with RNE)
                        nc.scalar.activation(
                            out=xhs[:, r0:r1, :], in_=xf[:, r0:r1, :],
                            func=IDENT, scale=64.0)
                        # xl8 = fp8((x*64) - fp16(x)*64), exact residual;
                        # lo weights are +-2^-8 so 64 * 2^-8 * sign == the
                        # +-0.25 hi scale: lo accumulates into the same psum
                        nc.vector.scalar_tensor_tensor(
                            out=xl8[:, b, 1 + r0:1 + r1, 1:1 + W],
                            in0=xf[:, r0:r1, :], scalar=64.0,
                            in1=xhs[:, r0:r1, :],
                            op0=Op.mult, op1=Op.subtract)
                xfs.append(xf)

            def conv_group(dst_y, wt, src_pad, b, two_pass, g0, gn):
                """conv1 group: 9 fp16 hi-shift MMs; if two_pass, the fp8
                residual (x64, weights +-2^-8) continues the same psum
                accumulation as 3 DoubleRow pairs + 3 regular MMs. The
                y slab is RNE(psum) via the exact ACT magic-constant pair."""
                ps = psum.tile([C, 4, BANK], F32, tag="psh")
                for k in range(gn):
                    r0 = (g0 + k) * HB
                    for s, (ky, kx) in enumerate(SHIFTS):
                        rh = src_pad[0][:, b, r0 + ky:r0 + ky + HB,
                                        kx:kx + W]
                        nc.tensor.matmul(
                            ps[:, k, 0:CHUNK], wt[:, s, :], rh,
                            start=(s == 0), stop=(s == 8 and not two_pass))
                    if two_pass and not lo8:
                        for s, (ky, kx) in enumerate(SHIFTS):
                            rl = xl16[:, b, r0 + ky:r0 + ky + HB, kx:kx + W]
                            nc.tensor.matmul(
                                ps[:, k, 0:CHUNK], wt[:, s, :], rl,
                                start=False, stop=(s == 8))
                    if two_pass and lo8:
                        for kx in range(3):
                            v0 = xl8[:, b, r0:r0 + HB, kx:kx + W]
                            pair = bass.AP(
                                tensor=v0.tensor, offset=v0.offset,
                                ap=[v0.ap[0], [WP8, 2], [WP8, HB], [1, W]])
                            nc.tensor.matmul(
                                ps[:, k, 0:CHUNK], w1d[:, kx, :, :], pair,
                                perf_mode=mybir.MatmulPerfMode.DoubleRow,
                                start=False, stop=False)
                        for kx in range(3):
                            rr = xl8[:, b, r0 + 2:r0 + 2 + HB, kx:kx + W]
                            nc.tensor.matmul(
                                ps[:, k, 0:CHUNK], w1r[:, kx, :], rr,
                                start=False, stop=(kx == 2))
                ys = dst_y[:, g0 * CHUNK:(g0 + gn) * CHUNK]
                nc.scalar.activation(out=ys, in_=ps[:, 0:gn, 0:CHUNK],
                                     func=IDENT, bias=MAGP)
                nc.scalar.activation(out=ys, in_=ys, func=IDENT, bias=MAGN)

            def conv(dst_y, wt, src_pad, b, two_pass):
                for g0, gn in GROUPS:
                    conv_group(dst_y, wt, src_pad, b, two_pass, g0, gn)

            def conv2_group(dst_y, wd, wr, src, b, g0, gn):
                """conv2: 3 DoubleRow pair-MMs (ky=0,1) + 3 regular (ky=2),
                all fp8, exact for integer x1."""
                ps = psum.tile([C, 4, BANK], F32, tag="psh")
                for k in range(gn):
                    r0 = (g0 + k) * HB
                    for kx in range(3):
                        v0 = src[:, b, r0:r0 + HB, kx:kx + W]
                        pair = bass.AP(
                            tensor=v0.tensor, offset=v0.offset,
                            ap=[v0.ap[0], [WP8, 2], [WP8, HB], [1, W]])
                        nc.tensor.matmul(
                            ps[:, k, 0:CHUNK], wd[:, kx, :, :], pair,
                            perf_mode=mybir.MatmulPerfMode.DoubleRow,
                            start=(kx == 0), stop=False)
                    for kx in range(3):
                        rr = src[:, b, r0 + 2:r0 + 2 + HB, kx:kx + W]
                        nc.tensor.matmul(
                            ps[:, k, 0:CHUNK], wr[:, kx, :], rr,
                            start=False, stop=(kx == 2))
                ys = dst_y[:, g0 * CHUNK:(g0 + gn) * CHUNK]
                nc.scalar.activation(out=ys, in_=ps[:, 0:gn, 0:CHUNK],
                                     func=IDENT, bias=MAGP)
                nc.scalar.activation(out=ys, in_=ys, func=IDENT, bias=MAGN)

            # ---- stage 1 ----
            def stage1(b):
                y = tmp.tile([C, H * W], F32, tag="y", bufs=2)
                conv(y, w1, (xh,), b, two_pass=two_pass1)
                # t = floor(y*sw1) = RNE(y*sw1 - 0.5)
                t = tmp.tile([C, H * W], I32, tag="t")
                nc.vector.tensor_scalar(out=t, in0=y[:], scalar1=A1,
                                        scalar2=0.5, op0=Op.mult,
                                        op1=Op.subtract)
                # ra = RNE(x*sc1 + bb1) + MAGIC   (ACT; fp32 add at ulp=1)
                ra = tmp.tile([C, H * W], F32, tag="ra")
                nc.scalar.activation(out=ra, in_=xfs[b][:], func=IDENT,
                                     bias=BB1C, scale=SC1)
                # v' = t*bw1 + ra  (= v + MAGIC, exact ints)
                vp = tmp.tile([C, H * W], F32, tag="vp")
                nc.vector.scalar_tensor_tensor(out=vp, in0=t[:], scalar=B1,
                                               in1=ra[:], op0=Op.mult,
                                               op1=Op.add)
                # u = RNE((v' - MAGIC)*s1)
                u = tmp.tile([C, H * W], I32, tag="u")
                nc.vector.tensor_scalar(out=u, in0=vp[:], scalar1=MAGIC,
                                        scalar2=S1, op0=Op.subtract,
                                        op1=Op.mult)
                # x1 = clip(u,-7,7) -> fp16 into padded buffer
                nc.vector.tensor_scalar(
                    out=x1p[:, b, 1:1 + H, 1:1 + W], in0=u[:],
                    scalar1=7.0, scalar2=-7.0, op0=Op.min, op1=Op.max)

            # ---- stage 2 (per-group elementwise to shorten the tail) ----
            def stage2(b, groups=GROUPS):
                y2 = tmp.tile([C, H * W], F32, tag="y", bufs=2)
                t2 = tmp.tile([C, H * W], I32, tag="t")
                r2s = tmp.tile([C, H * W], F32, tag="ra")
                u2 = tmp.tile([C, H * W], I32, tag="vp")
                for g0, gn in groups:
                    c0, c1 = g0 * CHUNK, (g0 + gn) * CHUNK
                    rr0, rr1 = g0 * HB, (g0 + gn) * HB
                    conv2_group(y2, w2d, w2r, x1p, b, g0, gn)
                    nc.vector.tensor_scalar(
                        out=t2[:, c0:c1], in0=y2[:, c0:c1], scalar1=A2,
                        scalar2=0.5, op0=Op.mult, op1=Op.subtract)
                    # r2s = x1*(sc2*s2) + bb2*s2   (ACT)
                    nc.scalar.activation(
                        out=r2s[:, c0:c1],
                        in_=x1p[:, b, 1 + rr0:1 + rr1, 1:1 + W],
                        func=IDENT, bias=RBI, scale=RSC)
                    # u2 = RNE(t2*(bw2*s2) + r2s)  (fused, host-verified)
                    nc.vector.scalar_tensor_tensor(
                        out=u2[:, c0:c1], in0=t2[:, c0:c1], scalar=B2S,
                        in1=r2s[:, c0:c1], op0=Op.mult, op1=Op.add)
                    nc.vector.tensor_scalar(
                        out=out_sb[:, b, rr0:rr1, :], in0=u2[:, c0:c1],
                        scalar1=7.0, scalar2=-7.0, op0=Op.min, op1=Op.max)
                    nc.sync.dma_start(out=d_o[:, b, rr0:rr1, :],
                                      in_=out_sb[:, b, rr0:rr1, :])

            # interleave so the PE instruction stream has ~2 images of
            # conv2 work between the last conv1 matmuls and conv2 of the
            # last image (whose x1 is produced by a serial DVE chain), and
            # finish with small groups to shorten the tail chain
            stage1(0)
            stage1(1)
            stage1(2)
            stage2(0)
            stage1(3)
            stage2(1)
            stage2(2)
            stage2(3, groups=[(0, 2), (2, 2), (4, 1), (5, 1), (6, 1)])

    nc.compile()
    _prog_cache[key] = nc
    return nc


# ---------------------------------------------------------------------------
# Entry point
# ---------------------------------------------------------------------------

last_results = None


def kernel(x, w1, w2, gamma1, beta1, mean1, var1,
           gamma2, beta2, mean2, var2):
    global last_results
    x, w1, w2 = np.asarray(x), np.asarray(w1), np.asarray(w2)
    gamma1, beta1, mean1, var1 = (np.asarray(a) for a in
                                  (gamma1, beta1, mean1, var1))
    gamma2, beta2, mean2, var2 = (np.asarray(a) for a in
                                  (gamma2, beta2, mean2, var2))
    w1t, w1d, w1r, w2d, w2r, cv = _host_prep(x, w1, w2, gamma1, beta1, mean1, var1,
                              gamma2, beta2, mean2, var2)
    nc = _build_program()

    in_maps = []
    for i in range(NCORES):
        shard = np.ascontiguousarray(
            x[i * BS:(i + 1) * BS].astype(f32).transpose(1, 0, 2, 3))
        in_maps.append({"xt": shard, "w1s": w1t, "w1d": w1d,
                        "w1r": w1r, "w2d": w2d, "w2r": w2r,
                        "cv": cv})

    trace = bool(int(os.environ.get("KERNEL_TRACE", "0")))
    kwargs = {}
    if trace:
        import concourse.bass_utils as _bu
        _bu.upload_artifacts = lambda tmpdir: ""
        kwargs["tmpdir"] = os.environ.get("KERNEL_TRACE_DIR", "/tmp/ktrace")
        os.makedirs(kwargs["tmpdir"], exist_ok=True)
    res = run_bass_kernel_spmd(nc, in_maps, core_ids=list(range(NCORES)),
                               trace=trace, **kwargs)
    last_results = res

    out = np.empty((B, C, H, W), np.float32)
    for i in range(NCORES):
        out[i * BS:(i + 1) * BS] = \
            res.results[i]["ot"].astype(np.float32).transpose(1, 0, 2, 3)
    return out



# revision 2
# speedup vs baseline: 1.2562x; 1.2562x over previous
"""Trainium2 Bass kernel for nn_BasicBlock_1w8a_q (quantized ResNet BasicBlock,
1-bit weights / 8-bit activations).

Strategy:
 - Pure data parallel over 8 NeuronCores: batch 32 -> 4 images per core.
 - Layout: channels C=128 on SBUF partitions, spatial on the free dim.
 - conv1 = fp16 hi pass (9 shifted matmuls) + e5m2 residual pass. The
   residual r = x - fp16(x) is representable directly in e5m2 (subnormals
   reach 2^-16), giving ~14-bit effective input precision; host-sim
   verified rel err 3.9e-3 vs the 2e-2 gate.
 - The residual and conv2 passes are fp8 and pair taps via DoubleRow.
   A duplicated +1-column-shifted copy of the fp8 image (pair stride 64)
   lets anti-diagonal tap pairs cover 9 taps in 4 DR + 1 regular matmul.
 - Host pre-computes: the fp16/e5m2 split (uploaded directly, no on-device
   split), and ra = bb1 + round(x*sc1) exactly as the reference (int16).
 - All rounds use hardware RNE output casts (ACT f32->i16 and DVE casts
   verified round-to-nearest-even on HW), so no magic-constant passes.
 - Elementwise chains run in 16-bit dtypes for 2x DVE throughput.
 - Data-dependent fused scales are grid-verified on host against the
   reference mapping before use (see _pick_scale/_pick_fused_stage2).
"""

import os

import numpy as np

import concourse.bass as bass
import concourse.bacc as bacc
import concourse.tile as tile
import concourse.mybir as mybir
from concourse.bass_utils import run_bass_kernel_spmd
from concourse.mybir import AluOpType as Op

F32 = mybir.dt.float32
F16 = mybir.dt.float16
I32 = mybir.dt.int32
I16 = mybir.dt.int16
I8 = mybir.dt.int8
F8 = mybir.dt.float8e4
F8E5 = mybir.dt.float8e5
IDENT = mybir.ActivationFunctionType.Identity

B, C, H, W = 32, 128, 56, 56
NCORES = 8
BS = B // NCORES            # images per core
HP, WP = H + 2, W + 2       # padded spatial
WP8 = 64                    # fp8 pair-buffer column pitch
HB = 8                      # output rows per psum chunk
NCH = H // HB               # chunks per image (7)
CHUNK = HB * W              # 448 columns per psum chunk
BANK = 512                  # fp32 slots per PSUM bank
GROUPS = [(0, 4), (4, 3)]   # (first chunk, n chunks) per psum group
SHIFTS = [(ky, kx) for ky in range(3) for kx in range(3)]
# DoubleRow pairs on the [y, copy, j] fp8 buffer (copy1[j] = padded[j+1]):
#   (tapA, tapB, pair-stride-in-elems): stride 64 = copy axis, 128 = next row
PAIRS = [((0, 0), (0, 1), 64), ((1, 0), (1, 1), 64),
         ((2, 0), (2, 1), 64), ((0, 2), (1, 2), 128)]
SINGLE = (2, 2)

f32 = np.float32


# ---------------------------------------------------------------------------
# Host-side prep: mirrors the reference's f32 op order exactly.
# ---------------------------------------------------------------------------

def _qfn(x, prec):
    n = f32(2.0 ** prec - 1.0)
    q = (np.round(x * n) / n).astype(f32)
    return (x + (q - x)).astype(f32)


def _my_quantize(x, prec):
    T = np.clip(np.max(np.abs(x)), f32(1e-10), f32(255.0)).astype(f32)
    return (_qfn((np.clip(x, -T, T) / T).astype(f32), prec) * T).astype(f32)


def _bn_consts(gamma, beta, mean, var):
    gamma, beta, mean, var = (a.astype(f32) for a in (gamma, beta, mean, var))
    std = np.sqrt(var + f32(1e-5)).astype(f32)
    w = (gamma / std).astype(f32)
    bq = (beta - w * mean).astype(f32)
    T_w = np.max(np.abs(w)).astype(f32)
    bw = (_qfn((np.clip(w, -T_w, T_w) / T_w).astype(f32), 3) * f32(7.0)).astype(f32)
    qb = _my_quantize(bq, 14)
    t = (qb * f32(7.0)).astype(f32)
    t = (t * f32(1023.0)).astype(f32)
    t = (t / f32(4032.0)).astype(f32)
    t = (t * f32(7.0)).astype(f32)
    t = (t / T_w).astype(f32)
    bb = np.round(t).astype(f32)
    return bw, bb, T_w


def _sc_th(T_w):
    a = (f32(1023.0) / f32(4032.0)).astype(f32)
    a = (a * f32(7.0)).astype(f32)
    sc = np.round((a / T_w).astype(f32)).astype(f32)
    b2 = (f32(7.0) * f32(1023.0)).astype(f32)
    b2 = (b2 / f32(4032.0)).astype(f32)
    b2 = (b2 * f32(7.0)).astype(f32)
    Th = np.round((b2 / T_w).astype(f32)).astype(f32)
    return sc, Th


def _ref_final_vec(k, Th):
    # reference: round(clip(k,-Th,Th)/Th*7.0) elementwise in f32
    kk = np.clip(k.astype(f32), -Th, Th).astype(f32)
    return np.round(((kk / Th).astype(f32) * f32(7.0)).astype(f32))


def _scale_cands(Th):
    base = f32(f32(7.0) / f32(Th))
    out = [base]
    up, dn = base, base
    for _ in range(8):
        up = np.nextafter(up, f32(np.inf), dtype=f32)
        dn = np.nextafter(dn, f32(-np.inf), dtype=f32)
        out += [up, dn]
    return out


def _pick_scale(Th):
    """s (f32) with clip(RNE(k*s),-7,7) == round(clip(k,-Th,Th)/Th*7) for all
    integer k (device RNE == np.round, verified on HW)."""
    kk = np.arange(-3000, 3001, dtype=f32)
    want = _ref_final_vec(kk, f32(Th))
    for s in _scale_cands(Th):
        got = np.clip(np.round((kk * s).astype(f32)), -7.0, 7.0)
        if np.array_equal(got, want):
            return f32(s)
    raise AssertionError(f"no matching scale for Th={Th}")


def _pick_fused_stage2(bw2, bb2, sc2, Th2):
    """Stage-2 fusion: u2 = RNE(t2*(bw2*s) + (x1*(sc2*s) + bb2*s)) must equal
    ref round(clip(v2)/Th2*7) (then clip +-7) for v2 = t2*bw2 + x1*sc2 + bb2.
    Returns (B2s, rscale, rbias, s) all f32, host-verified over a full grid
    with a tie-margin so ACT fma-vs-two-round ambiguity cannot flip a round.
    """
    t2g = np.arange(-640, 641, dtype=f32)[None, :, None]       # [1,T,1]
    x1g = np.arange(-7, 8, dtype=f32)[None, None, :]           # [1,1,15]
    bwc = bw2.astype(f32)[:, None, None]                       # [C,1,1]
    bbc = bb2.astype(f32)[:, None, None]
    v2 = (t2g * bwc + x1g * f32(sc2) + bbc).astype(f32)        # exact ints
    want = np.clip(_ref_final_vec(v2, f32(Th2)), -7.0, 7.0)
    base = f32(f32(7.0) / f32(Th2))
    for j in range(0, 60):
        s = f32(base * f32(1.0 + j * 2.0 ** -19))
        B2s = (bw2 * s).astype(f32)
        rscale = f32(f32(sc2) * s)
        rbias = (bb2 * s).astype(f32)
        # device sim (two-round form)
        r2s = ((x1g * rscale).astype(f32) + rbias[:, None, None]).astype(f32)
        dev = ((t2g * B2s[:, None, None]).astype(f32) + r2s).astype(f32)
        got = np.clip(np.round(dev), -7.0, 7.0)
        if not np.array_equal(got, want):
            continue
        # tie-margin: exact value far enough from half-integers (so device
        # fma-vs-two-round differences, bounded ~6e-6 abs in-range, cannot
        # flip a round) unless the result saturates either way
        z = (t2g.astype(np.float64) * B2s.astype(np.float64)[:, None, None]
             + x1g.astype(np.float64) * float(rscale)
             + rbias.astype(np.float64)[:, None, None])
        dist = np.abs(z - (np.floor(z) + 0.5))
        safe = (dist > 3e-5) | (np.abs(z) > 7.6)
        if bool(np.all(safe)):
            return B2s, rscale, rbias, f32(s)
    raise AssertionError(f"no verified fused scale for Th2={Th2}")


def _pair_wtiles(w, dtype, scale=0.25):
    """Weight tiles for the 4-DR + 1-single tap schedule.
    Returns wp [C_in, 4, 2, C_out], ws [C_in, C_out] in `dtype`."""
    npdt = mybir.dt.np(dtype)
    sg = (np.sign(w) * f32(scale)).astype(f32)      # [O, I, 3, 3]
    wp = np.empty((C, 4, 2, C), f32)
    for p, (ta, tb, _s) in enumerate(PAIRS):
        wp[:, p, 0, :] = sg[:, :, ta[0], ta[1]].T
        wp[:, p, 1, :] = sg[:, :, tb[0], tb[1]].T
    ws = sg[:, :, SINGLE[0], SINGLE[1]].T.copy()
    return wp.astype(npdt), ws.astype(npdt)


def _hi_wtiles(w):
    sg = (np.sign(w) * 0.25).astype(np.float16)     # [O, I, 3, 3]
    t = np.empty((C, 9, C), np.float16)             # [ci, s, co]
    for s, (ky, kx) in enumerate(SHIFTS):
        t[:, s, :] = sg[:, :, ky, kx].T
    return t


def _host_prep(x, w1, w2, g1, b1, m1, v1, g2, b2, m2, v2):
    w1 = w1.astype(f32)
    w2 = w2.astype(f32)
    sw1 = np.abs(w1).mean(axis=(1, 2, 3), dtype=np.float32).astype(f32)
    sw2 = np.abs(w2).mean(axis=(1, 2, 3), dtype=np.float32).astype(f32)
    bw1, bb1, Tw1 = _bn_consts(g1, b1, m1, v1)
    bw2, bb2, Tw2 = _bn_consts(g2, b2, m2, v2)
    sc1, Th1 = _sc_th(Tw1)
    sc2, Th2 = _sc_th(Tw2)
    s1 = _pick_scale(Th1)
    B2s, rscale, rbias, _s2 = _pick_fused_stage2(bw2, bb2, sc2, Th2)

    cv = np.zeros((C, 8), f32)
    cv[:, 0] = sw1                       # A1
    cv[:, 1] = bw1                       # B1
    cv[:, 2] = s1                        # S1 (broadcast)
    cv[:, 3] = sw2                       # A2
    cv[:, 4] = B2s                       # bw2 * s2
    cv[:, 5] = rscale                    # sc2 * s2 (broadcast)
    cv[:, 6] = rbias                     # bb2 * s2
    w1p, w1s = _pair_wtiles(w1, F8E5)
    w2p, w2s = _pair_wtiles(w2, F8)
    return _hi_wtiles(w1), w1p, w1s, w2p, w2s, cv, sc1, bb1


def _shard_inputs(x, sc1, bb1):
    """Per-core input tensors: xh fp16 [C,BS,H,W], xl pair-padded e5m2
    [C,BS,HP,2,WP8], ra int16 [C,BS,H,W] (= bb1 + round(x*sc1), exact)."""
    e5 = mybir.dt.np(F8E5)
    shards = []
    for i in range(NCORES):
        xs = np.ascontiguousarray(
            x[i * BS:(i + 1) * BS].astype(f32).transpose(1, 0, 2, 3))
        xh = xs.astype(np.float16)
        xl = (xs - xh.astype(f32)).astype(e5)
        XL = np.zeros((C, BS, HP, 2, WP8), e5)
        XL[:, :, 1:1 + H, 0, 1:1 + W] = xl
        XL[:, :, 1:1 + H, 1, 0:W] = xl
        ra = (np.round((xs * sc1).astype(f32))
              + bb1[:, None, None, None]).astype(np.int16)
        shards.append({"xh": xh, "xl": XL, "ra": ra})
    return shards


# ---------------------------------------------------------------------------
# Device program
# ---------------------------------------------------------------------------

_prog_cache = {}


def _build_program():
    key = "nc"
    if key in _prog_cache:
        return _prog_cache[key]
    nc = bacc.Bacc("TRN2", target_bir_lowering=False, debug=False,
                   num_devices=NCORES)
    d_xh = nc.dram_tensor("xh", [C, BS, H, W], F16, kind="ExternalInput").ap()
    d_xl = nc.dram_tensor("xl", [C, BS, HP, 2, WP8], F8E5,
                          kind="ExternalInput").ap()
    d_ra = nc.dram_tensor("ra", [C, BS, H, W], I16, kind="ExternalInput").ap()
    d_w1 = nc.dram_tensor("w1s", [C, 9, C], F16, kind="ExternalInput").ap()
    d_w1p = nc.dram_tensor("w1p", [C, 4, 2, C], F8E5, kind="ExternalInput").ap()
    d_w1r = nc.dram_tensor("w1r", [C, C], F8E5, kind="ExternalInput").ap()
    d_w2p = nc.dram_tensor("w2p", [C, 4, 2, C], F8, kind="ExternalInput").ap()
    d_w2r = nc.dram_tensor("w2r", [C, C], F8, kind="ExternalInput").ap()
    d_cv = nc.dram_tensor("cv", [C, 8], F32, kind="ExternalInput").ap()
    d_o = nc.dram_tensor("ot", [C, BS, H, W], I8, kind="ExternalOutput").ap()

    with tile.TileContext(nc) as tc:
        with tc.tile_pool(name="const", bufs=1) as const, \
             tc.tile_pool(name="pads", bufs=1) as pads, \
             tc.tile_pool(name="tmp", bufs=2) as tmp, \
             tc.tile_pool(name="outp", bufs=1) as outp, \
             tc.tile_pool(name="psum", bufs=2, space="PSUM") as psum:

            cv = const.tile([C, 8], F32)
            w1 = const.tile([C, 9, C], F16)
            w1p = const.tile([C, 4, 2, C], F8E5)
            w1r = const.tile([C, C], F8E5)
            w2p = const.tile([C, 4, 2, C], F8)
            w2r = const.tile([C, C], F8)

            A1, B1, S1, A2, B2S, RSC, RBI = (cv[:, i:i + 1] for i in range(7))

            xh = pads.tile([C, BS, HP, WP], F16)
            XL = pads.tile([C, BS, HP, 2, WP8], F8E5)
            RA = pads.tile([C, BS, H, W], I16)
            X1 = pads.tile([C, BS, HP, 2, WP8], F8)
            out_sb = outp.tile([C, BS, H, W], I8)

            # zero-fill padding borders (xh halo + X1 copy0 halo rows/cols;
            # X1 copy1 cols >= W never feed a matmul read, but the top/bottom
            # pad rows of both copies do)
            for b in range(BS):
                nc.gpsimd.memset(xh[:, b, 0, :], 0.0)
                nc.gpsimd.memset(xh[:, b, HP - 1, :], 0.0)
                nc.gpsimd.memset(xh[:, b, 1:HP - 1, 0], 0.0)
                nc.gpsimd.memset(xh[:, b, 1:HP - 1, WP - 1], 0.0)
                nc.gpsimd.memset(X1[:, b, 0, :, :], 0.0)
                nc.gpsimd.memset(X1[:, b, HP - 1, :, :], 0.0)
                nc.gpsimd.memset(X1[:, b, 1:HP - 1, 0, 0], 0.0)
                nc.gpsimd.memset(X1[:, b, 1:HP - 1, 0, 1 + W], 0.0)
                nc.gpsimd.memset(X1[:, b, 1:HP - 1, 1, W], 0.0)

            # ---- input DMA (pieces so matmuls start early) ----
            PIECES = ((0, 10), (10, 34), (34, H))
            nc.sync.dma_start(out=cv, in_=d_cv)
            nc.sync.dma_start(out=w1, in_=d_w1)
            for b in range(BS):
                for r0, r1 in PIECES:
                    nc.sync.dma_start(out=xh[:, b, 1 + r0:1 + r1, 1:1 + W],
                                      in_=d_xh[:, b, r0:r1, :])
                nc.sync.dma_start(out=XL[:, b], in_=d_xl[:, b])
                nc.sync.dma_start(out=RA[:, b], in_=d_ra[:, b])
                if b == 0:
                    nc.sync.dma_start(out=w1p, in_=d_w1p)
                    nc.sync.dma_start(out=w1r, in_=d_w1r)
                    nc.sync.dma_start(out=w2p, in_=d_w2p)
                    nc.sync.dma_start(out=w2r, in_=d_w2r)

            def pair_ap(src, b, tap, stride, r0):
                """Moving AP for a DR pair: base at tap (ky,kx) of chunk r0
                in the [y, copy, j] pair buffer, partner at +stride elems."""
                ky, kx = tap
                v0 = src[:, b, r0 + ky:r0 + ky + HB, 0, kx:kx + W]
                return bass.AP(
                    tensor=v0.tensor, offset=v0.offset,
                    ap=[v0.ap[0], [stride, 2], [2 * WP8, HB], [1, W]])

            def fp8_group(ps, wp, wr, src, b, g0, gn, start):
                """9-tap fp8 pass over a chunk group: 4 DR pairs + 1 single
                per chunk, accumulating into ps (start=False continues)."""
                for k in range(gn):
                    r0 = (g0 + k) * HB
                    for p, (ta, tb, stride) in enumerate(PAIRS):
                        nc.tensor.matmul(
                            ps[:, k, 0:CHUNK], wp[:, p, :, :],
                            pair_ap(src, b, ta, stride, r0),
                            perf_mode=mybir.MatmulPerfMode.DoubleRow,
                            start=(start and p == 0), stop=False)
                    ky, kx = SINGLE
                    rr = src[:, b, r0 + ky:r0 + ky + HB, 0, kx:kx + W]
                    nc.tensor.matmul(ps[:, k, 0:CHUNK], wr, rr,
                                     start=False, stop=True)

            def conv1_group(ps, b, g0, gn):
                for k in range(gn):
                    r0 = (g0 + k) * HB
                    for s, (ky, kx) in enumerate(SHIFTS):
                        rh = xh[:, b, r0 + ky:r0 + ky + HB, kx:kx + W]
                        nc.tensor.matmul(ps[:, k, 0:CHUNK], w1[:, s, :], rh,
                                         start=(s == 0), stop=False)
                fp8_group(ps, w1p, w1r, XL, b, g0, gn, start=False)

            # ---- stage 1 per group: psum -> x1 (into X1 pair buffer) ----
            def stage1_group(b, g0, gn):
                ps = psum.tile([C, 4, BANK], F32, tag="ps")
                conv1_group(ps, b, g0, gn)
                ncol = gn * CHUNK
                rr0, rr1 = g0 * HB, (g0 + gn) * HB
                y = tmp.tile([C, 4 * CHUNK], I16, tag="y")
                t = tmp.tile([C, 4 * CHUNK], I16, tag="t")
                vp = tmp.tile([C, 4 * CHUNK], F16, tag="vp")
                u = tmp.tile([C, 4 * CHUNK], I16, tag="u")
                # y = RNE(psum)  (ACT RNE output cast)
                nc.scalar.activation(out=y[:, 0:ncol], in_=ps[:, 0:gn, 0:CHUNK],
                                     func=IDENT)
                # t = floor(y*sw1) = RNE(y*sw1 - 0.5)
                nc.vector.tensor_scalar(out=t[:, 0:ncol], in0=y[:, 0:ncol],
                                        scalar1=A1, scalar2=0.5,
                                        op0=Op.mult, op1=Op.subtract)
                # vp = t*bw1 + ra  (exact small ints in f16)
                nc.vector.scalar_tensor_tensor(
                    out=vp[:, 0:ncol], in0=t[:, 0:ncol], scalar=B1,
                    in1=RA[:, b, rr0:rr1, :], op0=Op.mult, op1=Op.add)
                # u = RNE(vp*s1)
                nc.vector.tensor_scalar(out=u[:, 0:ncol], in0=vp[:, 0:ncol],
                                        scalar1=S1, scalar2=0.0,
                                        op0=Op.mult, op1=Op.add)
                # x1 = clip(u,-7,7) -> both copies of the fp8 pair buffer
                nc.vector.tensor_scalar(
                    out=X1[:, b, 1 + rr0:1 + rr1, 0, 1:1 + W],
                    in0=u[:, 0:ncol], scalar1=7.0, scalar2=-7.0,
                    op0=Op.min, op1=Op.max)
                nc.vector.tensor_scalar(
                    out=X1[:, b, 1 + rr0:1 + rr1, 1, 0:W],
                    in0=u[:, 0:ncol], scalar1=7.0, scalar2=-7.0,
                    op0=Op.min, op1=Op.max)

            # ---- stage 2 per group: conv2 psum -> int8 out + DMA ----
            def stage2_group(b, g0, gn):
                ps = psum.tile([C, 4, BANK], F32, tag="ps")
                fp8_group(ps, w2p, w2r, X1, b, g0, gn, start=True)
                ncol = gn * CHUNK
                rr0, rr1 = g0 * HB, (g0 + gn) * HB
                y2 = tmp.tile([C, 4 * CHUNK], I16, tag="y")
                t2 = tmp.tile([C, 4 * CHUNK], I16, tag="t")
                r2s = tmp.tile([C, 4 * CHUNK], F32, tag="r2s")
                u2 = tmp.tile([C, 4 * CHUNK], I16, tag="u")
                nc.scalar.activation(out=y2[:, 0:ncol],
                                     in_=ps[:, 0:gn, 0:CHUNK], func=IDENT)
                nc.vector.tensor_scalar(out=t2[:, 0:ncol], in0=y2[:, 0:ncol],
                                        scalar1=A2, scalar2=0.5,
                                        op0=Op.mult, op1=Op.subtract)
                # r2s = x1*(sc2*s2) + bb2*s2   (ACT, f32: fused-scale needs it)
                nc.scalar.activation(
                    out=r2s[:, 0:ncol],
                    in_=X1[:, b, 1 + rr0:1 + rr1, 0, 1:1 + W],
                    func=IDENT, bias=RBI, scale=RSC)
                # u2 = RNE(t2*(bw2*s2) + r2s)  (fused, host-verified)
                nc.vector.scalar_tensor_tensor(
                    out=u2[:, 0:ncol], in0=t2[:, 0:ncol], scalar=B2S,
                    in1=r2s[:, 0:ncol], op0=Op.mult, op1=Op.add)
                nc.vector.tensor_scalar(
                    out=out_sb[:, b, rr0:rr1, :], in0=u2[:, 0:ncol],
                    scalar1=7.0, scalar2=-7.0, op0=Op.min, op1=Op.max)
                nc.sync.dma_start(out=d_o[:, b, rr0:rr1, :],
                                  in_=out_sb[:, b, rr0:rr1, :])

            def stage1(b):
                for g0, gn in GROUPS:
                    stage1_group(b, g0, gn)

            def stage2(b, groups=GROUPS):
                for g0, gn in groups:
                    stage2_group(b, g0, gn)

            # interleave: keep ~2 images of conv work between the last conv1
            # matmuls and the dependent conv2; split the final image's groups
            # to shorten the serial elementwise tail
            stage1(0)
            stage1(1)
            stage1(2)
            stage2(0)
            stage1(3)
            stage2(1)
            stage2(2)
            stage2(3, groups=[(0, 2), (2, 2), (4, 1), (5, 1), (6, 1)])

    nc.compile()
    _prog_cache[key] = nc
    return nc


# ---------------------------------------------------------------------------
# Entry point
# ---------------------------------------------------------------------------

last_results = None


def kernel(x, w1, w2, gamma1, beta1, mean1, var1,
           gamma2, beta2, mean2, var2):
    global last_results
    x, w1, w2 = np.asarray(x), np.asarray(w1), np.asarray(w2)
    gamma1, beta1, mean1, var1 = (np.asarray(a) for a in
                                  (gamma1, beta1, mean1, var1))
    gamma2, beta2, mean2, var2 = (np.asarray(a) for a in
                                  (gamma2, beta2, mean2, var2))
    w1t, w1p, w1r, w2p, w2r, cv, sc1, bb1 = _host_prep(
        x, w1, w2, gamma1, beta1, mean1, var1, gamma2, beta2, mean2, var2)
    nc = _build_program()

    shards = _shard_inputs(x, sc1, bb1)
    in_maps = []
    for i in range(NCORES):
        m = dict(shards[i])
        m.update({"w1s": w1t, "w1p": w1p, "w1r": w1r,
                  "w2p": w2p, "w2r": w2r, "cv": cv})
        in_maps.append(m)

    trace = bool(int(os.environ.get("KERNEL_TRACE", "0")))
    kwargs = {}
    if trace:
        import concourse.bass_utils as _bu
        _bu.upload_artifacts = lambda tmpdir: ""
        kwargs["tmpdir"] = os.environ.get("KERNEL_TRACE_DIR", "/tmp/ktrace")
        os.makedirs(kwargs["tmpdir"], exist_ok=True)
    res = run_bass_kernel_spmd(nc, in_maps, core_ids=list(range(NCORES)),
                               trace=trace, **kwargs)
    last_results = res

    out = np.empty((B, C, H, W), np.float32)
    for i in range(NCORES):
        out[i * BS:(i + 1) * BS] = \
            res.results[i]["ot"].astype(np.float32).transpose(1, 0, 2, 3)
    return out


# revision 7
# speedup vs baseline: 1.3232x; 1.0533x over previous
"""Trainium2 Bass kernel for nn_BasicBlock_1w8a_q (quantized ResNet BasicBlock,
1-bit weights / 8-bit activations).

Strategy:
 - Pure data parallel over 8 NeuronCores: batch 32 -> 4 images per core.
 - Layout: channels C=128 on SBUF partitions, spatial on the free dim.
 - conv1 = fp16 hi pass (9 shifted matmuls) + e5m2 residual pass. The
   residual r = x - fp16(x) is representable directly in e5m2 (subnormals
   reach 2^-16), giving ~14-bit effective input precision; host-sim
   verified rel err 3.9e-3 vs the 2e-2 gate.
 - The residual and conv2 passes are fp8 and pair taps via DoubleRow.
   A duplicated +1-column-shifted copy of the fp8 image (pair stride 64)
   lets anti-diagonal tap pairs cover 9 taps in 4 DR + 1 regular matmul.
 - Host pre-computes: the fp16/e5m2 split (uploaded directly, no on-device
   split), and ra = bb1 + round(x*sc1) exactly as the reference (int16).
 - All rounds use hardware RNE output casts (ACT f32->i16 and DVE casts
   verified round-to-nearest-even on HW), so no magic-constant passes.
 - Elementwise chains run in 16-bit dtypes for 2x DVE throughput.
 - Data-dependent fused scales are grid-verified on host against the
   reference mapping before use (see _pick_scale/_pick_fused_stage2).
"""

import os

import numpy as np

import concourse.bass as bass
import concourse.bacc as bacc
import concourse.tile as tile
import concourse.mybir as mybir
from concourse.bass_utils import run_bass_kernel_spmd
from concourse.mybir import AluOpType as Op

F32 = mybir.dt.float32
F16 = mybir.dt.float16
I32 = mybir.dt.int32
I16 = mybir.dt.int16
I8 = mybir.dt.int8
F8 = mybir.dt.float8e4
F8E5 = mybir.dt.float8e5
IDENT = mybir.ActivationFunctionType.Identity

B, C, H, W = 32, 128, 56, 56
NCORES = 8
BS = B // NCORES            # images per core
HP, WP = H + 2, W + 2       # padded spatial
WP8 = 64                    # fp8 pair-buffer column pitch
HB = 8                      # output rows per psum chunk
NCH = H // HB               # chunks per image (7)
CHUNK = HB * W              # 448 columns per psum chunk
BANK = 512                  # fp32 slots per PSUM bank
GROUPS = [(0, 4), (4, 3)]   # (first chunk, n chunks) per psum group
SHIFTS = [(ky, kx) for ky in range(3) for kx in range(3)]
# DoubleRow pairs on the [y, copy, j] fp8 buffer (copy1[j] = padded[j+1]):
#   (tapA, tapB, pair-stride-in-elems): stride 64 = copy axis, 128 = next row
PAIRS = [((0, 0), (0, 1), 64), ((1, 0), (1, 1), 64),
         ((2, 0), (2, 1), 64), ((0, 2), (1, 2), 128)]
SINGLE = (2, 2)

f32 = np.float32


# ---------------------------------------------------------------------------
# Host-side prep: mirrors the reference's f32 op order exactly.
# ---------------------------------------------------------------------------

def _qfn(x, prec):
    n = f32(2.0 ** prec - 1.0)
    q = (np.round(x * n) / n).astype(f32)
    return (x + (q - x)).astype(f32)


def _my_quantize(x, prec):
    T = np.clip(np.max(np.abs(x)), f32(1e-10), f32(255.0)).astype(f32)
    return (_qfn((np.clip(x, -T, T) / T).astype(f32), prec) * T).astype(f32)


def _bn_consts(gamma, beta, mean, var):
    gamma, beta, mean, var = (a.astype(f32) for a in (gamma, beta, mean, var))
    std = np.sqrt(var + f32(1e-5)).astype(f32)
    w = (gamma / std).astype(f32)
    bq = (beta - w * mean).astype(f32)
    T_w = np.max(np.abs(w)).astype(f32)
    bw = (_qfn((np.clip(w, -T_w, T_w) / T_w).astype(f32), 3) * f32(7.0)).astype(f32)
    qb = _my_quantize(bq, 14)
    t = (qb * f32(7.0)).astype(f32)
    t = (t * f32(1023.0)).astype(f32)
    t = (t / f32(4032.0)).astype(f32)
    t = (t * f32(7.0)).astype(f32)
    t = (t / T_w).astype(f32)
    bb = np.round(t).astype(f32)
    return bw, bb, T_w


def _sc_th(T_w):
    a = (f32(1023.0) / f32(4032.0)).astype(f32)
    a = (a * f32(7.0)).astype(f32)
    sc = np.round((a / T_w).astype(f32)).astype(f32)
    b2 = (f32(7.0) * f32(1023.0)).astype(f32)
    b2 = (b2 / f32(4032.0)).astype(f32)
    b2 = (b2 * f32(7.0)).astype(f32)
    Th = np.round((b2 / T_w).astype(f32)).astype(f32)
    return sc, Th


def _ref_final_vec(k, Th):
    # reference: round(clip(k,-Th,Th)/Th*7.0) elementwise in f32
    kk = np.clip(k.astype(f32), -Th, Th).astype(f32)
    return np.round(((kk / Th).astype(f32) * f32(7.0)).astype(f32))


def _scale_cands(Th):
    base = f32(f32(7.0) / f32(Th))
    out = [base]
    up, dn = base, base
    for _ in range(8):
        up = np.nextafter(up, f32(np.inf), dtype=f32)
        dn = np.nextafter(dn, f32(-np.inf), dtype=f32)
        out += [up, dn]
    return out


def _pick_scale(Th):
    """s (f32) with clip(RNE(k*s),-7,7) == round(clip(k,-Th,Th)/Th*7) for all
    integer k (device RNE == np.round, verified on HW)."""
    kk = np.arange(-3000, 3001, dtype=f32)
    want = _ref_final_vec(kk, f32(Th))
    for s in _scale_cands(Th):
        got = np.clip(np.round((kk * s).astype(f32)), -7.0, 7.0)
        if np.array_equal(got, want):
            return f32(s)
    raise AssertionError(f"no matching scale for Th={Th}")


def _pick_fused_stage2(bw2, bb2, sc2, Th2):
    """Stage-2 fusion: u2 = RNE(t2*(bw2*s) + (x1*(sc2*s) + bb2*s)) must equal
    ref round(clip(v2)/Th2*7) (then clip +-7) for v2 = t2*bw2 + x1*sc2 + bb2.
    Returns (B2s, rscale, rbias, s) all f32, host-verified over a full grid
    with a tie-margin so ACT fma-vs-two-round ambiguity cannot flip a round.
    """
    t2g = np.arange(-640, 641, dtype=f32)[None, :, None]       # [1,T,1]
    x1g = np.arange(-7, 8, dtype=f32)[None, None, :]           # [1,1,15]
    bwc = bw2.astype(f32)[:, None, None]                       # [C,1,1]
    bbc = bb2.astype(f32)[:, None, None]
    v2 = (t2g * bwc + x1g * f32(sc2) + bbc).astype(f32)        # exact ints
    want = np.clip(_ref_final_vec(v2, f32(Th2)), -7.0, 7.0)
    base = f32(f32(7.0) / f32(Th2))
    for j in range(0, 60):
        s = f32(base * f32(1.0 + j * 2.0 ** -19))
        B2s = (bw2 * s).astype(f32)
        rscale = f32(f32(sc2) * s)
        rbias = (bb2 * s).astype(f32)
        # device sim (two-round form)
        r2s = ((x1g * rscale).astype(f32) + rbias[:, None, None]).astype(f32)
        dev = ((t2g * B2s[:, None, None]).astype(f32) + r2s).astype(f32)
        got = np.clip(np.round(dev), -7.0, 7.0)
        if not np.array_equal(got, want):
            continue
        # tie-margin: exact value far enough from half-integers (so device
        # fma-vs-two-round differences, bounded ~6e-6 abs in-range, cannot
        # flip a round) unless the result saturates either way
        z = (t2g.astype(np.float64) * B2s.astype(np.float64)[:, None, None]
             + x1g.astype(np.float64) * float(rscale)
             + rbias.astype(np.float64)[:, None, None])
        dist = np.abs(z - (np.floor(z) + 0.5))
        safe = (dist > 3e-5) | (np.abs(z) > 7.6)
        if bool(np.all(safe)):
            return B2s, rscale, rbias, f32(s)
    raise AssertionError(f"no verified fused scale for Th2={Th2}")


def _pair_wtiles(w, dtype, scale=0.25):
    """Weight tiles for the 4-DR + 1-single tap schedule.
    Returns wp [C_in, 4, 2, C_out], ws [C_in, C_out] in `dtype`."""
    npdt = mybir.dt.np(dtype)
    sg = (np.sign(w) * f32(scale)).astype(f32)      # [O, I, 3, 3]
    wp = np.empty((C, 4, 2, C), f32)
    for p, (ta, tb, _s) in enumerate(PAIRS):
        wp[:, p, 0, :] = sg[:, :, ta[0], ta[1]].T
        wp[:, p, 1, :] = sg[:, :, tb[0], tb[1]].T
    ws = sg[:, :, SINGLE[0], SINGLE[1]].T.copy()
    return wp.astype(npdt), ws.astype(npdt)


def _hi_wtiles(w):
    sg = (np.sign(w) * 0.25).astype(np.float16)     # [O, I, 3, 3]
    t = np.empty((C, 9, C), np.float16)             # [ci, s, co]
    for s, (ky, kx) in enumerate(SHIFTS):
        t[:, s, :] = sg[:, :, ky, kx].T
    return t


def _host_prep(x, w1, w2, g1, b1, m1, v1, g2, b2, m2, v2):
    w1 = w1.astype(f32)
    w2 = w2.astype(f32)
    sw1 = np.abs(w1).mean(axis=(1, 2, 3), dtype=np.float32).astype(f32)
    sw2 = np.abs(w2).mean(axis=(1, 2, 3), dtype=np.float32).astype(f32)
    bw1, bb1, Tw1 = _bn_consts(g1, b1, m1, v1)
    bw2, bb2, Tw2 = _bn_consts(g2, b2, m2, v2)
    sc1, Th1 = _sc_th(Tw1)
    sc2, Th2 = _sc_th(Tw2)
    s1 = _pick_scale(Th1)
    B2s, rscale, rbias, _s2 = _pick_fused_stage2(bw2, bb2, sc2, Th2)

    cv = np.zeros((C, 8), f32)
    cv[:, 0] = sw1                       # A1
    cv[:, 1] = bw1                       # B1
    cv[:, 2] = s1                        # S1 (broadcast)
    cv[:, 3] = sw2                       # A2
    cv[:, 4] = B2s                       # bw2 * s2
    cv[:, 5] = rscale                    # sc2 * s2 (broadcast)
    cv[:, 6] = rbias                     # bb2 * s2
    w1p, w1s = _pair_wtiles(w1, F8E5)
    w2p, w2s = _pair_wtiles(w2, F8)
    return _hi_wtiles(w1), w1p, w1s, w2p, w2s, cv, sc1, bb1


def _shard_inputs(x, sc1, bb1):
    """Per-core input tensors: xh padded fp16 [C,BS,HP,WP], xl pair-padded
    e5m2 [C,BS,HP,2,WP8], ra int8 [C,BS,H,W] (= bb1 + round(x*sc1), exact)."""
    e5 = mybir.dt.np(F8E5)
    shards = []
    for i in range(NCORES):
        xs = np.ascontiguousarray(
            x[i * BS:(i + 1) * BS].astype(f32).transpose(1, 0, 2, 3))
        xh = xs.astype(np.float16)
        XH = np.zeros((C, BS, HP, WP), np.float16)
        XH[:, :, 1:1 + H, 1:1 + W] = xh
        xl = (xs - xh.astype(f32)).astype(e5)
        XL = np.zeros((C, BS, HP, 2, WP8), e5)
        XL[:, :, 1:1 + H, 0, 1:1 + W] = xl
        XL[:, :, 1:1 + H, 1, 0:W] = xl
        ra = (np.round((xs * sc1).astype(f32))
              + bb1[:, None, None, None]).astype(f32)
        assert np.abs(ra).max() < 127, "ra overflows int8"
        shards.append({"xh": XH, "xl": XL, "ra": ra.astype(np.int8)})
    return shards


# ---------------------------------------------------------------------------
# Device program
# ---------------------------------------------------------------------------

_prog_cache = {}


def _build_program():
    key = "nc"
    if key in _prog_cache:
        return _prog_cache[key]
    nc = bacc.Bacc("TRN2", target_bir_lowering=False, debug=False,
                   num_devices=NCORES)
    d_xh = nc.dram_tensor("xh", [C, BS, HP, WP], F16, kind="ExternalInput").ap()
    d_xl = nc.dram_tensor("xl", [C, BS, HP, 2, WP8], F8E5,
                          kind="ExternalInput").ap()
    d_ra = nc.dram_tensor("ra", [C, BS, H, W], I8, kind="ExternalInput").ap()
    d_w1 = nc.dram_tensor("w1s", [C, 9, C], F16, kind="ExternalInput").ap()
    d_w1p = nc.dram_tensor("w1p", [C, 4, 2, C], F8E5, kind="ExternalInput").ap()
    d_w1r = nc.dram_tensor("w1r", [C, C], F8E5, kind="ExternalInput").ap()
    d_w2p = nc.dram_tensor("w2p", [C, 4, 2, C], F8, kind="ExternalInput").ap()
    d_w2r = nc.dram_tensor("w2r", [C, C], F8, kind="ExternalInput").ap()
    d_cv = nc.dram_tensor("cv", [C, 8], F32, kind="ExternalInput").ap()
    d_o = nc.dram_tensor("ot", [C, BS, H, W], I8, kind="ExternalOutput").ap()

    with tile.TileContext(nc) as tc:
        with tc.tile_pool(name="const", bufs=1) as const, \
             tc.tile_pool(name="pads", bufs=1) as pads, \
             tc.tile_pool(name="tmp", bufs=2) as tmp, \
             tc.tile_pool(name="outp", bufs=1) as outp, \
             tc.tile_pool(name="psum", bufs=2, space="PSUM") as psum:

            cv = const.tile([C, 8], F32)
            w1 = const.tile([C, 9, C], F16)
            w1p = const.tile([C, 4, 2, C], F8E5)
            w1r = const.tile([C, C], F8E5)
            w2p = const.tile([C, 4, 2, C], F8)
            w2r = const.tile([C, C], F8)

            A1, B1, S1, A2, B2S, RSC, RBI = (cv[:, i:i + 1] for i in range(7))

            xh = pads.tile([C, BS, HP, WP], F16)
            XL = pads.tile([C, BS, HP, 2, WP8], F8E5)
            RA = pads.tile([C, BS, H, W], I8)
            X1 = pads.tile([C, BS, HP, 2, WP8], F8)
            out_sb = outp.tile([C, BS, H, W], I8)

            # zero-fill X1 halo (copy0 halo rows/cols; X1 copy1 cols >= W
            # never feed a matmul read, but the top/bottom pad rows of both
            # copies do). xh/XL arrive pre-padded from the host.
            for b in range(BS):
                nc.gpsimd.memset(X1[:, b, 0, :, :], 0.0)
                nc.gpsimd.memset(X1[:, b, HP - 1, :, :], 0.0)
                nc.gpsimd.memset(X1[:, b, 1:HP - 1, 0, 0], 0.0)
                nc.gpsimd.memset(X1[:, b, 1:HP - 1, 0, 1 + W], 0.0)
                nc.gpsimd.memset(X1[:, b, 1:HP - 1, 1, W], 0.0)

            # ---- input DMA (pieces so matmuls start early) ----
            PIECES = ((0, 12), (12, 36), (36, HP))
            nc.sync.dma_start(out=w1, in_=d_w1)
            for b in range(BS):
                for r0, r1 in PIECES:
                    nc.sync.dma_start(out=xh[:, b, r0:r1, :],
                                      in_=d_xh[:, b, r0:r1, :])
                if b == 0:
                    nc.sync.dma_start(out=cv, in_=d_cv)
                nc.sync.dma_start(out=XL[:, b], in_=d_xl[:, b])
                nc.sync.dma_start(out=RA[:, b], in_=d_ra[:, b])
                if b == 0:
                    nc.sync.dma_start(out=w1p, in_=d_w1p)
                    nc.sync.dma_start(out=w1r, in_=d_w1r)
                    nc.sync.dma_start(out=w2p, in_=d_w2p)
                    nc.sync.dma_start(out=w2r, in_=d_w2r)

            def pair_ap(src, b, tap, stride, r0):
                """Moving AP for a DR pair: base at tap (ky,kx) of chunk r0
                in the [y, copy, j] pair buffer, partner at +stride elems."""
                ky, kx = tap
                v0 = src[:, b, r0 + ky:r0 + ky + HB, 0, kx:kx + W]
                return bass.AP(
                    tensor=v0.tensor, offset=v0.offset,
                    ap=[v0.ap[0], [stride, 2], [2 * WP8, HB], [1, W]])

            def fp8_group(ps, wp, wr, src, b, g0, gn, start):
                """9-tap fp8 pass over a chunk group: 4 DR pairs + 1 single
                per chunk, accumulating into ps (start=False continues)."""
                for k in range(gn):
                    r0 = (g0 + k) * HB
                    for p, (ta, tb, stride) in enumerate(PAIRS):
                        nc.tensor.matmul(
                            ps[:, k, 0:CHUNK], wp[:, p, :, :],
                            pair_ap(src, b, ta, stride, r0),
                            perf_mode=mybir.MatmulPerfMode.DoubleRow,
                            start=(start and p == 0), stop=False)
                    ky, kx = SINGLE
                    rr = src[:, b, r0 + ky:r0 + ky + HB, 0, kx:kx + W]
                    nc.tensor.matmul(ps[:, k, 0:CHUNK], wr, rr,
                                     start=False, stop=True)

            def conv1_group(ps, b, g0, gn):
                for k in range(gn):
                    r0 = (g0 + k) * HB
                    for s, (ky, kx) in enumerate(SHIFTS):
                        rh = xh[:, b, r0 + ky:r0 + ky + HB, kx:kx + W]
                        nc.tensor.matmul(ps[:, k, 0:CHUNK], w1[:, s, :], rh,
                                         start=(s == 0), stop=False)
                fp8_group(ps, w1p, w1r, XL, b, g0, gn, start=False)

            # ---- stage 1 per group: psum -> x1 (into X1 pair buffer) ----
            def stage1_group(b, g0, gn):
                ps = psum.tile([C, 4, BANK], F32, tag="ps")
                conv1_group(ps, b, g0, gn)
                ncol = gn * CHUNK
                rr0, rr1 = g0 * HB, (g0 + gn) * HB
                y = tmp.tile([C, 4 * CHUNK], I16, tag="y")
                t = tmp.tile([C, 4 * CHUNK], I16, tag="t")
                vp = tmp.tile([C, 4 * CHUNK], F16, tag="vp")
                u = tmp.tile([C, 4 * CHUNK], I16, tag="u")
                # y = RNE(psum)  (ACT RNE output cast)
                nc.scalar.activation(out=y[:, 0:ncol], in_=ps[:, 0:gn, 0:CHUNK],
                                     func=IDENT)
                # t = floor(y*sw1) = RNE(y*sw1 - 0.5)
                nc.vector.tensor_scalar(out=t[:, 0:ncol], in0=y[:, 0:ncol],
                                        scalar1=A1, scalar2=0.5,
                                        op0=Op.mult, op1=Op.subtract)
                # vp = t*bw1 + ra  (exact small ints in f16)
                nc.vector.scalar_tensor_tensor(
                    out=vp[:, 0:ncol], in0=t[:, 0:ncol], scalar=B1,
                    in1=RA[:, b, rr0:rr1, :], op0=Op.mult, op1=Op.add)
                # u = RNE(vp*s1)
                nc.vector.tensor_scalar(out=u[:, 0:ncol], in0=vp[:, 0:ncol],
                                        scalar1=S1, scalar2=0.0,
                                        op0=Op.mult, op1=Op.add)
                # x1 = clip(u,-7,7) -> both copies of the fp8 pair buffer
                nc.vector.tensor_scalar(
                    out=X1[:, b, 1 + rr0:1 + rr1, 0, 1:1 + W],
                    in0=u[:, 0:ncol], scalar1=7.0, scalar2=-7.0,
                    op0=Op.min, op1=Op.max)
                nc.vector.tensor_scalar(
                    out=X1[:, b, 1 + rr0:1 + rr1, 1, 0:W],
                    in0=u[:, 0:ncol], scalar1=7.0, scalar2=-7.0,
                    op0=Op.min, op1=Op.max)

            # ---- stage 2 per group: conv2 psum -> int8 out + DMA ----
            def stage2_group(b, g0, gn):
                ps = psum.tile([C, 4, BANK], F32, tag="ps")
                ncol = gn * CHUNK
                rr0, rr1 = g0 * HB, (g0 + gn) * HB
                y2 = tmp.tile([C, 4 * CHUNK], I16, tag="y")
                t2 = tmp.tile([C, 4 * CHUNK], I16, tag="t")
                r2s = tmp.tile([C, 4 * CHUNK], F32, tag="r2s")
                u2 = tmp.tile([C, 4 * CHUNK], I16, tag="u")
                # r2s = x1*(sc2*s2) + bb2*s2 (ACT, f32: fused-scale needs it).
                # Issued before the matmuls: it depends only on X1, so the
                # ACT slot runs under the conv2 matmuls instead of after y2.
                nc.scalar.activation(
                    out=r2s[:, 0:ncol],
                    in_=X1[:, b, 1 + rr0:1 + rr1, 0, 1:1 + W],
                    func=IDENT, bias=RBI, scale=RSC)
                fp8_group(ps, w2p, w2r, X1, b, g0, gn, start=True)
                nc.scalar.activation(out=y2[:, 0:ncol],
                                     in_=ps[:, 0:gn, 0:CHUNK], func=IDENT)
                nc.vector.tensor_scalar(out=t2[:, 0:ncol], in0=y2[:, 0:ncol],
                                        scalar1=A2, scalar2=0.5,
                                        op0=Op.mult, op1=Op.subtract)
                # u2 = RNE(t2*(bw2*s2) + r2s)  (fused, host-verified)
                nc.vector.scalar_tensor_tensor(
                    out=u2[:, 0:ncol], in0=t2[:, 0:ncol], scalar=B2S,
                    in1=r2s[:, 0:ncol], op0=Op.mult, op1=Op.add)
                nc.vector.tensor_scalar(
                    out=out_sb[:, b, rr0:rr1, :], in0=u2[:, 0:ncol],
                    scalar1=7.0, scalar2=-7.0, op0=Op.min, op1=Op.max)
                nc.sync.dma_start(out=d_o[:, b, rr0:rr1, :],
                                  in_=out_sb[:, b, rr0:rr1, :])

            def stage1(b):
                for g0, gn in GROUPS:
                    stage1_group(b, g0, gn)

            def stage2(b, groups=GROUPS):
                for g0, gn in groups:
                    stage2_group(b, g0, gn)

            # interleave: keep ~2 images of conv work between the last conv1
            # matmuls and the dependent conv2; split the final image's groups
            # to shorten the serial elementwise tail
            stage1(0)
            stage1(1)
            stage1(2)
            stage2(0)
            stage1(3)
            stage2(1)
            stage2(2)
            stage2(3, groups=[(k, 1) for k in range(NCH)])

    nc.compile()
    _prog_cache[key] = nc
    return nc


# ---------------------------------------------------------------------------
# Entry point
# ---------------------------------------------------------------------------

last_results = None


def kernel(x, w1, w2, gamma1, beta1, mean1, var1,
           gamma2, beta2, mean2, var2):
    global last_results
    x, w1, w2 = np.asarray(x), np.asarray(w1), np.asarray(w2)
    gamma1, beta1, mean1, var1 = (np.asarray(a) for a in
                                  (gamma1, beta1, mean1, var1))
    gamma2, beta2, mean2, var2 = (np.asarray(a) for a in
                                  (gamma2, beta2, mean2, var2))
    w1t, w1p, w1r, w2p, w2r, cv, sc1, bb1 = _host_prep(
        x, w1, w2, gamma1, beta1, mean1, var1, gamma2, beta2, mean2, var2)
    nc = _build_program()

    shards = _shard_inputs(x, sc1, bb1)
    in_maps = []
    for i in range(NCORES):
        m = dict(shards[i])
        m.update({"w1s": w1t, "w1p": w1p, "w1r": w1r,
                  "w2p": w2p, "w2r": w2r, "cv": cv})
        in_maps.append(m)

    trace = bool(int(os.environ.get("KERNEL_TRACE", "0")))
    kwargs = {}
    if trace:
        import concourse.bass_utils as _bu
        _bu.upload_artifacts = lambda tmpdir: ""
        kwargs["tmpdir"] = os.environ.get("KERNEL_TRACE_DIR", "/tmp/ktrace")
        os.makedirs(kwargs["tmpdir"], exist_ok=True)
    res = run_bass_kernel_spmd(nc, in_maps, core_ids=list(range(NCORES)),
                               trace=trace, **kwargs)
    last_results = res

    out = np.empty((B, C, H, W), np.float32)
    for i in range(NCORES):
        out[i * BS:(i + 1) * BS] = \
            res.results[i]["ot"].astype(np.float32).transpose(1, 0, 2, 3)
    return out


# revision 10
# speedup vs baseline: 1.3713x; 1.0363x over previous
"""Trainium2 Bass kernel for nn_BasicBlock_1w8a_q (quantized ResNet BasicBlock,
1-bit weights / 8-bit activations).

Strategy:
 - Pure data parallel over 8 NeuronCores: batch 32 -> 4 images per core.
 - Layout: channels C=128 on SBUF partitions, spatial on the free dim.
 - conv1 = fp16 hi pass (9 shifted matmuls) + e5m2 residual pass. The
   residual r = x - fp16(x) is representable directly in e5m2 (subnormals
   reach 2^-16), giving ~14-bit effective input precision; host-sim
   verified rel err 3.9e-3 vs the 2e-2 gate.
 - The residual and conv2 passes are fp8 and pair taps via DoubleRow.
   A duplicated +1-column-shifted copy of the fp8 image (pair stride 64)
   lets anti-diagonal tap pairs cover 9 taps in 4 DR + 1 regular matmul.
 - Host pre-computes: the fp16/e5m2 split (uploaded directly, no on-device
   split), and ra = bb1 + round(x*sc1) exactly as the reference (int16).
 - All rounds use hardware RNE output casts (ACT f32->i16 and DVE casts
   verified round-to-nearest-even on HW), so no magic-constant passes.
 - Elementwise chains run in 16-bit dtypes for 2x DVE throughput.
 - Data-dependent fused scales are grid-verified on host against the
   reference mapping before use (see _pick_scale/_pick_fused_stage2).
"""

import os

import numpy as np

import concourse.bass as bass
import concourse.bacc as bacc
import concourse.tile as tile
import concourse.mybir as mybir
from concourse.bass_utils import run_bass_kernel_spmd
from concourse.mybir import AluOpType as Op

F32 = mybir.dt.float32
F16 = mybir.dt.float16
I32 = mybir.dt.int32
I16 = mybir.dt.int16
I8 = mybir.dt.int8
F8 = mybir.dt.float8e4
F8E5 = mybir.dt.float8e5
IDENT = mybir.ActivationFunctionType.Identity

B, C, H, W = 32, 128, 56, 56
NCORES = 8
BS = B // NCORES            # images per core
HP, WP = H + 2, W + 2       # padded spatial
WP8 = 64                    # fp8 pair-buffer column pitch
HB = 8                      # output rows per psum chunk
NCH = H // HB               # chunks per image (7)
CHUNK = HB * W              # 448 columns per psum chunk
BANK = 512                  # fp32 slots per PSUM bank
GROUPS = [(0, 2), (2, 2), (4, 2), (6, 1)]   # (first chunk, n chunks)
GB = 2                      # psum banks per group tile
SHIFTS = [(ky, kx) for ky in range(3) for kx in range(3)]
# DoubleRow pairs on the [y, copy, j] fp8 buffer (copy1[j] = padded[j+1]):
#   (tapA, tapB, pair-stride-in-elems): stride 64 = copy axis, 128 = next row
PAIRS = [((0, 0), (0, 1), 64), ((1, 0), (1, 1), 64),
         ((2, 0), (2, 1), 64), ((0, 2), (1, 2), 128)]
SINGLE = (2, 2)

f32 = np.float32


# ---------------------------------------------------------------------------
# Host-side prep: mirrors the reference's f32 op order exactly.
# ---------------------------------------------------------------------------

def _qfn(x, prec):
    n = f32(2.0 ** prec - 1.0)
    q = (np.round(x * n) / n).astype(f32)
    return (x + (q - x)).astype(f32)


def _my_quantize(x, prec):
    T = np.clip(np.max(np.abs(x)), f32(1e-10), f32(255.0)).astype(f32)
    return (_qfn((np.clip(x, -T, T) / T).astype(f32), prec) * T).astype(f32)


def _bn_consts(gamma, beta, mean, var):
    gamma, beta, mean, var = (a.astype(f32) for a in (gamma, beta, mean, var))
    std = np.sqrt(var + f32(1e-5)).astype(f32)
    w = (gamma / std).astype(f32)
    bq = (beta - w * mean).astype(f32)
    T_w = np.max(np.abs(w)).astype(f32)
    bw = (_qfn((np.clip(w, -T_w, T_w) / T_w).astype(f32), 3) * f32(7.0)).astype(f32)
    qb = _my_quantize(bq, 14)
    t = (qb * f32(7.0)).astype(f32)
    t = (t * f32(1023.0)).astype(f32)
    t = (t / f32(4032.0)).astype(f32)
    t = (t * f32(7.0)).astype(f32)
    t = (t / T_w).astype(f32)
    bb = np.round(t).astype(f32)
    return bw, bb, T_w


def _sc_th(T_w):
    a = (f32(1023.0) / f32(4032.0)).astype(f32)
    a = (a * f32(7.0)).astype(f32)
    sc = np.round((a / T_w).astype(f32)).astype(f32)
    b2 = (f32(7.0) * f32(1023.0)).astype(f32)
    b2 = (b2 / f32(4032.0)).astype(f32)
    b2 = (b2 * f32(7.0)).astype(f32)
    Th = np.round((b2 / T_w).astype(f32)).astype(f32)
    return sc, Th


def _ref_final_vec(k, Th):
    # reference: round(clip(k,-Th,Th)/Th*7.0) elementwise in f32
    kk = np.clip(k.astype(f32), -Th, Th).astype(f32)
    return np.round(((kk / Th).astype(f32) * f32(7.0)).astype(f32))


def _scale_cands(Th):
    base = f32(f32(7.0) / f32(Th))
    out = [base]
    up, dn = base, base
    for _ in range(8):
        up = np.nextafter(up, f32(np.inf), dtype=f32)
        dn = np.nextafter(dn, f32(-np.inf), dtype=f32)
        out += [up, dn]
    return out


def _pick_scale(Th):
    """s (f32) with clip(RNE(k*s),-7,7) == round(clip(k,-Th,Th)/Th*7) for all
    integer k (device RNE == np.round, verified on HW)."""
    kk = np.arange(-3000, 3001, dtype=f32)
    want = _ref_final_vec(kk, f32(Th))
    for s in _scale_cands(Th):
        got = np.clip(np.round((kk * s).astype(f32)), -7.0, 7.0)
        if np.array_equal(got, want):
            return f32(s)
    raise AssertionError(f"no matching scale for Th={Th}")


def _pick_fused_stage2(bw2, bb2, sc2, Th2):
    """Stage-2 fusion: u2 = RNE(t2*(bw2*s) + (x1*(sc2*s) + bb2*s)) must equal
    ref round(clip(v2)/Th2*7) (then clip +-7) for v2 = t2*bw2 + x1*sc2 + bb2.
    Returns (B2s, rscale, rbias, s) all f32, host-verified over a full grid
    with a tie-margin so ACT fma-vs-two-round ambiguity cannot flip a round.
    """
    t2g = np.arange(-640, 641, dtype=f32)[None, :, None]       # [1,T,1]
    x1g = np.arange(-7, 8, dtype=f32)[None, None, :]           # [1,1,15]
    bwc = bw2.astype(f32)[:, None, None]                       # [C,1,1]
    bbc = bb2.astype(f32)[:, None, None]
    v2 = (t2g * bwc + x1g * f32(sc2) + bbc).astype(f32)        # exact ints
    want = np.clip(_ref_final_vec(v2, f32(Th2)), -7.0, 7.0)
    base = f32(f32(7.0) / f32(Th2))
    for j in range(0, 60):
        s = f32(base * f32(1.0 + j * 2.0 ** -19))
        B2s = (bw2 * s).astype(f32)
        rscale = f32(f32(sc2) * s)
        rbias = (bb2 * s).astype(f32)
        # device sim (two-round form)
        r2s = ((x1g * rscale).astype(f32) + rbias[:, None, None]).astype(f32)
        dev = ((t2g * B2s[:, None, None]).astype(f32) + r2s).astype(f32)
        got = np.clip(np.round(dev), -7.0, 7.0)
        if not np.array_equal(got, want):
            continue
        # tie-margin: exact value far enough from half-integers (so device
        # fma-vs-two-round differences, bounded ~6e-6 abs in-range, cannot
        # flip a round) unless the result saturates either way
        z = (t2g.astype(np.float64) * B2s.astype(np.float64)[:, None, None]
             + x1g.astype(np.float64) * float(rscale)
             + rbias.astype(np.float64)[:, None, None])
        dist = np.abs(z - (np.floor(z) + 0.5))
        safe = (dist > 3e-5) | (np.abs(z) > 7.6)
        if bool(np.all(safe)):
            return B2s, rscale, rbias, f32(s)
    raise AssertionError(f"no verified fused scale for Th2={Th2}")


def _pair_wtiles(w, dtype, scale=0.25):
    """Weight tiles for the 4-DR + 1-single tap schedule.
    Returns wp [C_in, 4, 2, C_out], ws [C_in, C_out] in `dtype`."""
    npdt = mybir.dt.np(dtype)
    sg = (np.sign(w) * f32(scale)).astype(f32)      # [O, I, 3, 3]
    wp = np.empty((C, 4, 2, C), f32)
    for p, (ta, tb, _s) in enumerate(PAIRS):
        wp[:, p, 0, :] = sg[:, :, ta[0], ta[1]].T
        wp[:, p, 1, :] = sg[:, :, tb[0], tb[1]].T
    ws = sg[:, :, SINGLE[0], SINGLE[1]].T.copy()
    return wp.astype(npdt), ws.astype(npdt)


def _hi_wtiles(w):
    sg = (np.sign(w) * 0.25).astype(np.float16)     # [O, I, 3, 3]
    t = np.empty((C, 9, C), np.float16)             # [ci, s, co]
    for s, (ky, kx) in enumerate(SHIFTS):
        t[:, s, :] = sg[:, :, ky, kx].T
    return t


def _host_prep(x, w1, w2, g1, b1, m1, v1, g2, b2, m2, v2):
    w1 = w1.astype(f32)
    w2 = w2.astype(f32)
    sw1 = np.abs(w1).mean(axis=(1, 2, 3), dtype=np.float32).astype(f32)
    sw2 = np.abs(w2).mean(axis=(1, 2, 3), dtype=np.float32).astype(f32)
    bw1, bb1, Tw1 = _bn_consts(g1, b1, m1, v1)
    bw2, bb2, Tw2 = _bn_consts(g2, b2, m2, v2)
    sc1, Th1 = _sc_th(Tw1)
    sc2, Th2 = _sc_th(Tw2)
    s1 = _pick_scale(Th1)
    B2s, rscale, rbias, _s2 = _pick_fused_stage2(bw2, bb2, sc2, Th2)

    cv = np.zeros((C, 8), f32)
    cv[:, 0] = sw1                       # A1
    cv[:, 1] = bw1                       # B1
    cv[:, 2] = s1                        # S1 (broadcast)
    cv[:, 3] = sw2                       # A2
    cv[:, 4] = B2s                       # bw2 * s2
    cv[:, 5] = rscale                    # sc2 * s2 (broadcast)
    cv[:, 6] = rbias                     # bb2 * s2
    w1p, w1s = _pair_wtiles(w1, F8E5)
    w2p, w2s = _pair_wtiles(w2, F8)
    return _hi_wtiles(w1), w1p, w1s, w2p, w2s, cv, sc1, bb1


def _shard_inputs(x, sc1, bb1):
    """Per-core input tensors: xh padded fp16 [C,BS,HP,WP], xl pair-padded
    e5m2 [C,BS,HP,2,WP8], ra int8 [C,BS,H,W] (= bb1 + round(x*sc1), exact)."""
    e5 = mybir.dt.np(F8E5)
    shards = []
    for i in range(NCORES):
        xs = np.ascontiguousarray(
            x[i * BS:(i + 1) * BS].astype(f32).transpose(1, 0, 2, 3))
        xh = xs.astype(np.float16)
        XH = np.zeros((C, BS, HP, WP), np.float16)
        XH[:, :, 1:1 + H, 1:1 + W] = xh
        xl = (xs - xh.astype(f32)).astype(e5)
        XL = np.zeros((C, BS, HP, 2, WP8), e5)
        XL[:, :, 1:1 + H, 0, 1:1 + W] = xl
        XL[:, :, 1:1 + H, 1, 0:W] = xl
        ra = (np.round((xs * sc1).astype(f32))
              + bb1[:, None, None, None]).astype(f32)
        assert np.abs(ra).max() < 127, "ra overflows int8"
        shards.append({"xh": XH, "xl": XL, "ra": ra.astype(np.int8)})
    return shards


# ---------------------------------------------------------------------------
# Device program
# ---------------------------------------------------------------------------

_prog_cache = {}


def _build_program():
    key = "nc"
    if key in _prog_cache:
        return _prog_cache[key]
    nc = bacc.Bacc("TRN2", target_bir_lowering=False, debug=False,
                   num_devices=NCORES)
    d_xh = nc.dram_tensor("xh", [C, BS, HP, WP], F16, kind="ExternalInput").ap()
    d_xl = nc.dram_tensor("xl", [C, BS, HP, 2, WP8], F8E5,
                          kind="ExternalInput").ap()
    d_ra = nc.dram_tensor("ra", [C, BS, H, W], I8, kind="ExternalInput").ap()
    d_w1 = nc.dram_tensor("w1s", [C, 9, C], F16, kind="ExternalInput").ap()
    d_w1p = nc.dram_tensor("w1p", [C, 4, 2, C], F8E5, kind="ExternalInput").ap()
    d_w1r = nc.dram_tensor("w1r", [C, C], F8E5, kind="ExternalInput").ap()
    d_w2p = nc.dram_tensor("w2p", [C, 4, 2, C], F8, kind="ExternalInput").ap()
    d_w2r = nc.dram_tensor("w2r", [C, C], F8, kind="ExternalInput").ap()
    d_cv = nc.dram_tensor("cv", [C, 8], F32, kind="ExternalInput").ap()
    d_o = nc.dram_tensor("ot", [C, BS, H, W], I8, kind="ExternalOutput").ap()

    with tile.TileContext(nc) as tc:
        with tc.tile_pool(name="const", bufs=1) as const, \
             tc.tile_pool(name="pads", bufs=1) as pads, \
             tc.tile_pool(name="tmp", bufs=2) as tmp, \
             tc.tile_pool(name="outp", bufs=1) as outp, \
             tc.tile_pool(name="psum", bufs=4, space="PSUM") as psum:

            cv = const.tile([C, 8], F32)
            w1 = const.tile([C, 9, C], F16)
            w1p = const.tile([C, 4, 2, C], F8E5)
            w1r = const.tile([C, C], F8E5)
            w2p = const.tile([C, 4, 2, C], F8)
            w2r = const.tile([C, C], F8)

            A1, B1, S1, A2, B2S, RSC, RBI = (cv[:, i:i + 1] for i in range(7))

            xh = pads.tile([C, BS, HP, WP], F16)
            XL = pads.tile([C, BS, HP, 2, WP8], F8E5)
            RA = pads.tile([C, BS, H, W], I8)
            X1 = pads.tile([C, BS, HP, 2, WP8], F8)
            out_sb = outp.tile([C, BS, H, W], I8)

            # PE warm-up: garbage matmuls while input DMA is in flight so
            # the HAM clock gate reaches 8/8 before the first real matmul.
            scr_w = const.tile([C, C], F16)
            scr_x = const.tile([C, CHUNK], F16)
            nc.gpsimd.memset(scr_w, 1.0)
            nc.gpsimd.memset(scr_x, 1.0)
            ps_w = psum.tile([C, GB, BANK], F32, tag="ps")
            for _ in range(12):
                nc.tensor.matmul(ps_w[:, 0, 0:CHUNK], scr_w, scr_x,
                                 start=True, stop=True)

            # zero-fill X1 halo (copy0 halo rows/cols; X1 copy1 cols >= W
            # never feed a matmul read, but the top/bottom pad rows of both
            # copies do). xh/XL arrive pre-padded from the host.
            for b in range(BS):
                nc.gpsimd.memset(X1[:, b, 0, :, :], 0.0)
                nc.gpsimd.memset(X1[:, b, HP - 1, :, :], 0.0)
                nc.gpsimd.memset(X1[:, b, 1:HP - 1, 0, 0], 0.0)
                nc.gpsimd.memset(X1[:, b, 1:HP - 1, 0, 1 + W], 0.0)
                nc.gpsimd.memset(X1[:, b, 1:HP - 1, 1, W], 0.0)

            # ---- input DMA (pieces so matmuls start early) ----
            PIECES = ((0, 12), (12, 36), (36, HP))
            nc.sync.dma_start(out=w1, in_=d_w1)
            for b in range(BS):
                for r0, r1 in PIECES:
                    nc.sync.dma_start(out=xh[:, b, r0:r1, :],
                                      in_=d_xh[:, b, r0:r1, :])
                if b == 0:
                    nc.sync.dma_start(out=cv, in_=d_cv)
                nc.sync.dma_start(out=XL[:, b], in_=d_xl[:, b])
                nc.sync.dma_start(out=RA[:, b], in_=d_ra[:, b])
                if b == 0:
                    nc.sync.dma_start(out=w1p, in_=d_w1p)
                    nc.sync.dma_start(out=w1r, in_=d_w1r)
                    nc.sync.dma_start(out=w2p, in_=d_w2p)
                    nc.sync.dma_start(out=w2r, in_=d_w2r)

            def pair_ap(src, b, tap, stride, r0):
                """Moving AP for a DR pair: base at tap (ky,kx) of chunk r0
                in the [y, copy, j] pair buffer, partner at +stride elems."""
                ky, kx = tap
                v0 = src[:, b, r0 + ky:r0 + ky + HB, 0, kx:kx + W]
                return bass.AP(
                    tensor=v0.tensor, offset=v0.offset,
                    ap=[v0.ap[0], [stride, 2], [2 * WP8, HB], [1, W]])

            def fp8_group(ps, wp, wr, src, b, g0, gn, start):
                """9-tap fp8 pass over a chunk group: 4 DR pairs + 1 single
                per chunk, accumulating into ps (start=False continues)."""
                for k in range(gn):
                    r0 = (g0 + k) * HB
                    for p, (ta, tb, stride) in enumerate(PAIRS):
                        nc.tensor.matmul(
                            ps[:, k, 0:CHUNK], wp[:, p, :, :],
                            pair_ap(src, b, ta, stride, r0),
                            perf_mode=mybir.MatmulPerfMode.DoubleRow,
                            start=(start and p == 0), stop=False)
                    ky, kx = SINGLE
                    rr = src[:, b, r0 + ky:r0 + ky + HB, 0, kx:kx + W]
                    nc.tensor.matmul(ps[:, k, 0:CHUNK], wr, rr,
                                     start=False, stop=True)

            def conv1_group(ps, b, g0, gn):
                for k in range(gn):
                    r0 = (g0 + k) * HB
                    for s, (ky, kx) in enumerate(SHIFTS):
                        rh = xh[:, b, r0 + ky:r0 + ky + HB, kx:kx + W]
                        nc.tensor.matmul(ps[:, k, 0:CHUNK], w1[:, s, :], rh,
                                         start=(s == 0), stop=False)
                fp8_group(ps, w1p, w1r, XL, b, g0, gn, start=False)

            # ---- stage 1 per group: psum -> x1 (into X1 pair buffer) ----
            def stage1_group(b, g0, gn):
                ps = psum.tile([C, GB, BANK], F32, tag="ps")
                conv1_group(ps, b, g0, gn)
                ncol = gn * CHUNK
                rr0, rr1 = g0 * HB, (g0 + gn) * HB
                y = tmp.tile([C, GB * CHUNK], I16, tag="y")
                t = tmp.tile([C, GB * CHUNK], I16, tag="t")
                vp = tmp.tile([C, GB * CHUNK], F16, tag="vp")
                u = tmp.tile([C, GB * CHUNK], I16, tag="u")
                # y = RNE(psum)  (ACT RNE output cast)
                nc.scalar.activation(out=y[:, 0:ncol], in_=ps[:, 0:gn, 0:CHUNK],
                                     func=IDENT)
                # t = floor(y*sw1) = RNE(y*sw1 - 0.5)
                nc.vector.tensor_scalar(out=t[:, 0:ncol], in0=y[:, 0:ncol],
                                        scalar1=A1, scalar2=0.5,
                                        op0=Op.mult, op1=Op.subtract)
                # vp = t*bw1 + ra  (exact small ints in f16)
                nc.vector.scalar_tensor_tensor(
                    out=vp[:, 0:ncol], in0=t[:, 0:ncol], scalar=B1,
                    in1=RA[:, b, rr0:rr1, :], op0=Op.mult, op1=Op.add)
                # u = RNE(vp*s1)
                nc.vector.tensor_scalar(out=u[:, 0:ncol], in0=vp[:, 0:ncol],
                                        scalar1=S1, scalar2=0.0,
                                        op0=Op.mult, op1=Op.add)
                # x1 = clip(u,-7,7) -> both copies of the fp8 pair buffer
                nc.vector.tensor_scalar(
                    out=X1[:, b, 1 + rr0:1 + rr1, 0, 1:1 + W],
                    in0=u[:, 0:ncol], scalar1=7.0, scalar2=-7.0,
                    op0=Op.min, op1=Op.max)
                nc.vector.tensor_scalar(
                    out=X1[:, b, 1 + rr0:1 + rr1, 1, 0:W],
                    in0=u[:, 0:ncol], scalar1=7.0, scalar2=-7.0,
                    op0=Op.min, op1=Op.max)

            # ---- stage 2 per group: conv2 psum -> int8 out + DMA ----
            def stage2_group(b, g0, gn):
                ps = psum.tile([C, GB, BANK], F32, tag="ps")
                ncol = gn * CHUNK
                rr0, rr1 = g0 * HB, (g0 + gn) * HB
                y2 = tmp.tile([C, GB * CHUNK], I16, tag="y")
                t2 = tmp.tile([C, GB * CHUNK], I16, tag="t")
                r2s = tmp.tile([C, GB * CHUNK], F32, tag="r2s")
                u2 = tmp.tile([C, GB * CHUNK], I16, tag="u")
                # r2s = x1*(sc2*s2) + bb2*s2 (ACT, f32: fused-scale needs it).
                # Issued before the matmuls: it depends only on X1, so the
                # ACT slot runs under the conv2 matmuls instead of after y2.
                nc.scalar.activation(
                    out=r2s[:, 0:ncol],
                    in_=X1[:, b, 1 + rr0:1 + rr1, 0, 1:1 + W],
                    func=IDENT, bias=RBI, scale=RSC)
                fp8_group(ps, w2p, w2r, X1, b, g0, gn, start=True)
                nc.scalar.activation(out=y2[:, 0:ncol],
                                     in_=ps[:, 0:gn, 0:CHUNK], func=IDENT)
                nc.vector.tensor_scalar(out=t2[:, 0:ncol], in0=y2[:, 0:ncol],
                                        scalar1=A2, scalar2=0.5,
                                        op0=Op.mult, op1=Op.subtract)
                # u2 = RNE(t2*(bw2*s2) + r2s)  (fused, host-verified)
                nc.vector.scalar_tensor_tensor(
                    out=u2[:, 0:ncol], in0=t2[:, 0:ncol], scalar=B2S,
                    in1=r2s[:, 0:ncol], op0=Op.mult, op1=Op.add)
                nc.vector.tensor_scalar(
                    out=out_sb[:, b, rr0:rr1, :], in0=u2[:, 0:ncol],
                    scalar1=7.0, scalar2=-7.0, op0=Op.min, op1=Op.max)
                nc.sync.dma_start(out=d_o[:, b, rr0:rr1, :],
                                  in_=out_sb[:, b, rr0:rr1, :])

            def stage1(b):
                for g0, gn in GROUPS:
                    stage1_group(b, g0, gn)

            def stage2(b, groups=GROUPS):
                for g0, gn in groups:
                    stage2_group(b, g0, gn)

            # interleave: keep ~2 images of conv work between the last conv1
            # matmuls and the dependent conv2; split the final image's groups
            # to shorten the serial elementwise tail
            stage1(0)
            stage1(1)
            stage1(2)
            stage2(0)
            stage1(3)
            stage2(1)
            stage2(2)
            stage2(3, groups=[(0, 2), (2, 2), (4, 1), (5, 1), (6, 1)])

    nc.compile()
    _prog_cache[key] = nc
    return nc


# ---------------------------------------------------------------------------
# Entry point
# ---------------------------------------------------------------------------

last_results = None


def kernel(x, w1, w2, gamma1, beta1, mean1, var1,
           gamma2, beta2, mean2, var2):
    global last_results
    x, w1, w2 = np.asarray(x), np.asarray(w1), np.asarray(w2)
    gamma1, beta1, mean1, var1 = (np.asarray(a) for a in
                                  (gamma1, beta1, mean1, var1))
    gamma2, beta2, mean2, var2 = (np.asarray(a) for a in
                                  (gamma2, beta2, mean2, var2))
    w1t, w1p, w1r, w2p, w2r, cv, sc1, bb1 = _host_prep(
        x, w1, w2, gamma1, beta1, mean1, var1, gamma2, beta2, mean2, var2)
    nc = _build_program()

    shards = _shard_inputs(x, sc1, bb1)
    in_maps = []
    for i in range(NCORES):
        m = dict(shards[i])
        m.update({"w1s": w1t, "w1p": w1p, "w1r": w1r,
                  "w2p": w2p, "w2r": w2r, "cv": cv})
        in_maps.append(m)

    trace = bool(int(os.environ.get("KERNEL_TRACE", "0")))
    kwargs = {}
    if trace:
        import concourse.bass_utils as _bu
        _bu.upload_artifacts = lambda tmpdir: ""
        kwargs["tmpdir"] = os.environ.get("KERNEL_TRACE_DIR", "/tmp/ktrace")
        os.makedirs(kwargs["tmpdir"], exist_ok=True)
    res = run_bass_kernel_spmd(nc, in_maps, core_ids=list(range(NCORES)),
                               trace=trace, **kwargs)
    last_results = res

    out = np.empty((B, C, H, W), np.float32)
    for i in range(NCORES):
        out[i * BS:(i + 1) * BS] = \
            res.results[i]["ot"].astype(np.float32).transpose(1, 0, 2, 3)
    return out
